# revision 9
# baseline (speedup 1.0000x reference)
"""Trainium2 Bass kernel for nn_MultiHeadAttention_81999515616076.

Reference computation (per batch b):
    xn = LN(x)                                    [N, IN]
    q  = xn @ W_q   -> [N, H, D]
    k,v= xn @ W_kv  -> [N, H, D] each
    ckv= LN(c_emb) @ W_ctx + b_ctx -> ck, cv      [M, D] (shared across heads)
    keys per head = [self keys (N)] + [null key] + [ctx keys (M)]  (2177 total)
    out = softmax(q.k / sqrt(D)) @ values         [N, H, D]
    y  = LN(out.reshape(N, H*D) @ W_out)          [N, IN]

Sharding (8 cores): core c -> batch b = c//4, head group g = c%4 (heads 4g..4g+3).
Per-core: LN+transpose of x, fp32r projections, flash-style attention for its 4
heads (scores computed transposed: [keys, tokens]; softmax denominator via a
ones-column in the PV matmul; no max subtraction -- scores are bounded ~N(0,0.4)),
out-projection partials, per-128-token-tile ReduceScatter(add) over the 4 cores
of each batch, and final LN (entirely on the Pool engine) on the received rows.
Host only slices inputs / concatenates outputs.

Perf notes vs the original baseline (427964 ns):
  - exp weights + PV operands in bf16 (halves ACT write traffic + SBUF reads).
  - softmax normalize via DMA-broadcast of the reciprocal row (no PE broadcast
    matmuls, no PSUM->SBUF casts on DVE).
  - ReduceScatter chunked per 128-token tile so the tail collective is 256KB.
  - final LN on Pool (scalar_tensor_tensor with accum stats), keeping the DVE
    and ACT queues free of collective-dependent work.
  - startup: x DMAs own the sync queue; constants via memset/gpsimd queue.
"""

import sys

sys.path.insert(0, "/opt/trn_rl_repo")

import numpy as np

import concourse.bacc as bacc
import concourse.tile as tile
import concourse.mybir as mybir

B, N, IN = 2, 2048, 1024
H, D = 16, 64
CTX_DIM, M_CTX = 768, 128
NCORES = 8
HG = 4               # heads per core
FH = HG * D          # 256 local head-feats
BLK = 512            # token block
NBLK = N // BLK      # 4
NT = N // 128        # 16 128-token tiles
KT = 17              # 16 self key tiles + 1 ctx key tile (null key handled separately)
SCALE = D ** -0.5    # 0.125
EPS = 1e-5

f32 = mybir.dt.float32
f32r = mybir.dt.float32r
bf16 = mybir.dt.bfloat16
i32 = mybir.dt.int32
AF = mybir.ActivationFunctionType
OP = mybir.AluOpType


def build_program():
    nc = bacc.Bacc("TRN2", target_bir_lowering=False, debug=False, num_devices=NCORES)

    # ---- per-core DRAM tensors (values sharded by host) ----
    x_d = nc.dram_tensor("x_loc", [N, IN], f32, kind="ExternalInput")
    wq_d = nc.dram_tensor("wq_loc", [IN, FH], f32, kind="ExternalInput")
    wk_d = nc.dram_tensor("wk_loc", [IN, FH], f32, kind="ExternalInput")
    wv_d = nc.dram_tensor("wv_loc", [IN, FH], f32, kind="ExternalInput")
    wout_d = nc.dram_tensor("wout_loc", [FH, IN], f32, kind="ExternalInput")
    wctx_d = nc.dram_tensor("wctx", [CTX_DIM, 2 * D], f32, kind="ExternalInput")
    cemb_d = nc.dram_tensor("cemb_loc", [M_CTX, CTX_DIM], f32, kind="ExternalInput")
    nullkv_d = nc.dram_tensor("nullkv", [2, D], f32, kind="ExternalInput")
    lng_d = nc.dram_tensor("ln_g", [IN], f32, kind="ExternalInput")
    lnb_d = nc.dram_tensor("ln_b", [IN], f32, kind="ExternalInput")
    ctxg_d = nc.dram_tensor("ctx_g", [CTX_DIM], f32, kind="ExternalInput")
    ctxb_d = nc.dram_tensor("ctx_b", [CTX_DIM], f32, kind="ExternalInput")
    bctx_d = nc.dram_tensor("b_ctx", [2 * D], f32, kind="ExternalInput")
    outg_d = nc.dram_tensor("out_g", [IN], f32, kind="ExternalInput")
    outb_d = nc.dram_tensor("out_b", [IN], f32, kind="ExternalInput")
    ident_d = nc.dram_tensor("const_ident", [128, 128], f32, kind="ExternalInput")
    y_out_d = nc.dram_tensor("y_out", [BLK, IN], f32, kind="ExternalOutput")
    # internal DRAM for the collective (per 128-token tile to minimize the tail).
    # Partials travel in bf16 to nearly halve collective time.
    ypart_d = [nc.dram_tensor(f"y_partial{t}", [128, IN], bf16) for t in range(NT)]
    yred_d = [nc.dram_tensor(f"y_red{t}", [32, IN], bf16) for t in range(NT)]
    # bounce buffer for the softmax-denominator partition broadcast
    dden_d = nc.dram_tensor("dden", [16, BLK], f32)
    warm_in_d = nc.dram_tensor("warm_in", [128, 8], bf16)
    warm_out_d = nc.dram_tensor("warm_out", [32, 8], bf16)

    with tile.TileContext(nc) as tc:
        _emit(nc, tc, locals())
    nc.compile()
    return nc


def _emit(nc, tc, t):
    from contextlib import ExitStack

    x_d, cemb_d = t["x_d"], t["cemb_d"]
    wq_d, wk_d, wv_d, wout_d, wctx_d = t["wq_d"], t["wk_d"], t["wv_d"], t["wout_d"], t["wctx_d"]
    nullkv_d, bctx_d = t["nullkv_d"], t["bctx_d"]
    lng_d, lnb_d, ctxg_d, ctxb_d = t["lng_d"], t["lnb_d"], t["ctxg_d"], t["ctxb_d"]
    outg_d, outb_d = t["outg_d"], t["outb_d"]
    y_out_d, ypart_d, yred_d = t["y_out_d"], t["ypart_d"], t["yred_d"]
    ident_d, dden_d = t["ident_d"], t["dden_d"]

    with ExitStack() as ctx:
        persist = ctx.enter_context(tc.tile_pool(name="persist", bufs=1))
        stat = ctx.enter_context(tc.tile_pool(name="stat", bufs=4))

        # ---------------- Phase 0: constants & first x tiles ----------------
        # Tiny ReduceScatter to absorb the ~2x cold-start cost of the first
        # collective while phase 1 runs (result unused).
        nc.gpsimd.collective_compute(
            "ReduceScatter",
            OP.add,
            replica_groups=[[0, 1, 2, 3], [4, 5, 6, 7]],
            ins=[t["warm_in_d"].ap()],
            outs=[t["warm_out_d"].ap()],
        )
        # constants that need no DMA: memsets (cheap, no queue dependencies)
        eps_t = persist.tile([128, 1], f32, name="eps", tag="eps")
        nc.vector.memset(eps_t, EPS)
        ones_f = persist.tile([1, 128], f32, name="ones_f", tag="ones_f")
        nc.vector.memset(ones_f, 1.0)
        ones_r = ones_f.bitcast(f32r)

        # small const DMAs on the gpsimd queue (sync queue is reserved for x)
        ident = persist.tile([128, 128], f32r, name="ident", tag="ident")
        nc.gpsimd.dma_start(ident, ident_d.ap().bitcast(f32r))
        g_sb = persist.tile([128, 8], f32, name="g_sb", tag="g_sb")
        nc.gpsimd.dma_start(g_sb, lng_d.ap().rearrange("(c p) -> p c", p=128))
        lnb_sb = persist.tile([128, 8], f32r, name="lnb_sb", tag="lnb_sb")
        nc.gpsimd.dma_start(lnb_sb, lnb_d.ap().rearrange("(c p) -> p c", p=128).bitcast(f32r))
        ctxg_sb = persist.tile([128, 6], f32, name="ctxg_sb", tag="ctxg_sb")
        nc.gpsimd.dma_start(ctxg_sb, ctxg_d.ap().rearrange("(c p) -> p c", p=128))
        ctxb_sb = persist.tile([128, 6], f32r, name="ctxb_sb", tag="ctxb_sb")
        nc.gpsimd.dma_start(ctxb_sb, ctxb_d.ap().rearrange("(c p) -> p c", p=128).bitcast(f32r))

        # null key/value: knull2 rows 0:64 and 64:128 both = null_k (for the two
        # row-packed head positions); nullv2 = [null_v | 1] in bf16 (PV operand).
        knull2 = persist.tile([128, 1], f32r, name="knull2", tag="knull2")
        nk_ap = nullkv_d.ap()[0:1, :].rearrange("a b -> b a").bitcast(f32r)
        nc.gpsimd.dma_start(knull2[0:64, :], nk_ap)
        nc.gpsimd.dma_start(knull2[64:128, :], nk_ap)
        nullv_f32 = stat.tile([1, 64], f32, name="nullv_f32", tag="nullv_f32")
        nc.gpsimd.dma_start(nullv_f32, nullkv_d.ap()[1:2, :])
        nullv2 = persist.tile([1, 65], bf16, name="nullv2", tag="nullv2")
        nc.vector.tensor_copy(nullv2[0:1, 0:64], nullv_f32)
        nc.vector.memset(nullv2[0:1, 64:65], 1.0)

        # Heavy P0 (weights + context projection), emitted AFTER block-0's LN/transpose
        # chains so the first x tiles hit the DMA queue first.
        wq_sb, wk_sb, wv_sb, wctx_sb, wout_sb = [], [], [], [], []
        cb_q, cb_k = [], []
        cv_row = persist.tile([1, FH], f32r, name="cv_row", tag="cv_row")
        ckvT_sb = persist.tile([128, M_CTX], f32r, name="ckvT", tag="ckvT")
        ck2 = persist.tile([128, M_CTX], f32r, name="ck2", tag="ck2")
        cv_ext = persist.tile([128, 65], bf16, name="cv_ext", tag="cv_ext")

        def emit_p0_heavy(p0sb, psP, psT):
            # weight DMAs split between scalar & gpsimd queues; scaling on gpsimd
            for name, dram, lst, eng in (
                ("wq", wq_d, wq_sb, nc.scalar), ("wk", wk_d, wk_sb, nc.scalar),
                ("wv", wv_d, wv_sb, nc.gpsimd),
            ):
                for c in range(8):
                    w = persist.tile([128, FH], f32r, name=f"{name}{c}", tag=f"{name}{c}")
                    eng.dma_start(w, dram.ap()[128 * c : 128 * (c + 1), :].bitcast(f32r))
                    nc.gpsimd.tensor_scalar_mul(w, w, g_sb[:, c : c + 1])
                    lst.append(w)
            for c in range(6):
                w = persist.tile([128, 2 * D], f32r, name=f"wctx{c}", tag=f"wctx{c}")
                nc.gpsimd.dma_start(w, wctx_d.ap()[128 * c : 128 * (c + 1), :].bitcast(f32r))
                nc.gpsimd.tensor_scalar_mul(w, w, ctxg_sb[:, c : c + 1])
                wctx_sb.append(w)
            for c in range(2):
                w = persist.tile([128, IN], f32r, name=f"wout{c}", tag=f"wout{c}")
                nc.scalar.dma_start(w, wout_d.ap()[128 * c : 128 * (c + 1), :].bitcast(f32r))
                wout_sb.append(w)
            # LN-beta folded biases: cb[j] = (ln_b @ W')[128j:128j+128] as [128,1]
            for wsb, lst in ((wq_sb, cb_q), (wk_sb, cb_k)):
                for j in range(2):
                    ps = psP.tile([128, 1], f32, name="p0bias", tag="proj")
                    for c in range(8):
                        nc.tensor.matmul(ps, wsb[c][:, 128 * j : 128 * (j + 1)].bitcast(f32),
                                         lnb_sb[:, c : c + 1].bitcast(f32), start=(c == 0), stop=(c == 7))
                    cb = persist.tile([128, 1], f32, name=f"cb{len(lst)}_{id(wsb) % 97}", tag=f"cb{len(cb_q)}_{len(cb_k)}")
                    nc.vector.tensor_copy(cb, ps)
                    lst.append(cb)
            # v bias as a row [1, FH] (added via a K=1 ones matmul)
            psc = psP.tile([1, FH], f32, name="p0cv", tag="proj")
            for c in range(8):
                nc.tensor.matmul(psc, lnb_sb[:, c : c + 1], wv_sb[c], start=(c == 0), stop=(c == 7))
            nc.vector.tensor_copy(cv_row, psc)
            # ---- context projection: ckv^T = W_ctx'.T @ LN(c_emb).T + bias ----
            cemb_sb = p0sb.tile([128, CTX_DIM], f32, name="cemb", tag="cemb")
            nc.gpsimd.dma_start(cemb_sb, cemb_d.ap())
            stc = stat.tile([128, 3, 6], f32, name="stc", tag="stc")
            for i in range(3):
                nc.vector.bn_stats(stc[:, i, :], cemb_sb[:, 256 * i : 256 * (i + 1)])
            mvc = stat.tile([128, 2], f32, name="mvc", tag="mvc")
            nc.vector.bn_aggr(mvc, stc)
            rstd_c = stat.tile([128, 1], f32, name="rstd_c", tag="rstd_c")
            lnv = stat.tile([128, 1], f32, name="lnv", tag="lnv")
            nc.scalar.activation(lnv, mvc[:, 1:2], AF.Sqrt, bias=eps_t[:, 0:1])
            nc.vector.reciprocal_approx_fast(rstd_c, lnv)
            zc = p0sb.tile([128, CTX_DIM], f32r, name="zc", tag="zc")
            nc.vector.tensor_scalar(zc, cemb_sb, mvc[:, 0:1], rstd_c, op0=OP.subtract, op1=OP.mult)
            tpc = psT.tile([128, CTX_DIM], f32r, name="tpc", tag="tp")
            for c in range(6):
                nc.tensor.transpose(tpc[:, 128 * c : 128 * (c + 1)], zc[:, 128 * c : 128 * (c + 1)], ident)
            zcT = p0sb.tile([128, 6, 128], f32r, name="zcT", tag="zcT")
            nc.any.tensor_copy(zcT, tpc.rearrange("p (c w) -> p c w", c=6))
            # bias = (ctx_b @ W_ctx')^T + b_ctx
            psb2 = psP.tile([128, 1], f32, name="p0bias2", tag="proj")
            for c in range(6):
                nc.tensor.matmul(psb2, wctx_sb[c].bitcast(f32), ctxb_sb[:, c : c + 1].bitcast(f32),
                                 start=(c == 0), stop=(c == 5))
            bctx_sb = stat.tile([128, 1], f32, name="bctx_sb", tag="bctx_sb")
            nc.gpsimd.dma_start(bctx_sb, bctx_d.ap().rearrange("(a p) -> p a", p=128))
            ckv_bias = stat.tile([128, 1], f32, name="ckv_bias", tag="ckv_bias")
            nc.vector.tensor_tensor(ckv_bias, psb2, bctx_sb, op=OP.add)
            psk = psP.tile([128, M_CTX], f32, name="psk", tag="proj")
            for c in range(6):
                nc.tensor.matmul(psk, wctx_sb[c], zcT[:, c, :], start=(c == 0), stop=(c == 5))
            nc.scalar.activation(ckvT_sb, psk, AF.Identity, bias=ckv_bias)
            # ck duplicated into both row-halves (for 2-head row packing)
            nc.sync.dma_start(ck2[0:64, :], ckvT_sb[0:64, :])
            nc.sync.dma_start(ck2[64:128, :], ckvT_sb[0:64, :])
            # cv in normal layout [M_CTX, 64] with a ones column -> [128, 65] bf16
            cvT_tmp = p0sb.tile([64, M_CTX], f32r, name="cvT_tmp", tag="cvT_tmp")
            nc.sync.dma_start(cvT_tmp, ckvT_sb[64:128, :])
            ps_cv = psT.tile([128, 64], f32r, name="ps_cv", tag="tp")
            nc.tensor.transpose(ps_cv, cvT_tmp, ident[0:64, 0:64])
            nc.any.tensor_copy(cv_ext[:, 0:64], ps_cv)
            nc.vector.memset(cv_ext[:, 64:65], 1.0)

        # ---------------- persistent activation tensors ----------------
        qT = [persist.tile([128, N], f32r, name=f"qT{j}", tag=f"qT{j}") for j in range(2)]
        kT = [persist.tile([128, N], f32r, name=f"kT{j}", tag=f"kT{j}") for j in range(2)]
        attnT = [persist.tile([128, N], f32r, name=f"attnT{j}", tag=f"attnT{j}") for j in range(2)]
        v_tiles = []
        for i in range(16):
            vt = persist.tile([128, HG, 65], bf16, name=f"v{i}", tag=f"v{i}")
            nc.gpsimd.memset(vt[:, :, 64:65], 1.0)
            v_tiles.append(vt)

        # ---------------- Phase 1: LN(x), transpose, q/k/v projections ----------------
        with tc.tile_pool(name="xp", bufs=3) as xp, \
             tc.tile_pool(name="zp", bufs=2) as zp, \
             tc.tile_pool(name="ztp", bufs=2) as ztp, \
             tc.tile_pool(name="p0sb", bufs=2) as p0sb, \
             tc.tile_pool(name="tpp", bufs=2, space="PSUM") as tpp, \
             tc.tile_pool(name="projp", bufs=2, space="PSUM") as projp, \
             tc.tile_pool(name="vpp", bufs=2, space="PSUM") as vpp:

            def emit_tts(blk):
                zT = ztp.tile([128, 8, BLK], f32r, name="zT", tag="zT")
                for tt in range(4):
                    t0 = BLK * blk + 128 * tt
                    x_t = xp.tile([128, IN], f32, name="x_t", tag="x_t")
                    nc.sync.dma_start(x_t, x_d.ap()[t0 : t0 + 128, :])
                    st = stat.tile([128, 2, 6], f32, name="st", tag="st")
                    nc.vector.bn_stats(st[:, 0, :], x_t[:, 0:512])
                    nc.vector.bn_stats(st[:, 1, :], x_t[:, 512:1024])
                    mv = stat.tile([128, 2], f32, name="mv", tag="mv")
                    nc.vector.bn_aggr(mv, st)
                    lv = stat.tile([128, 1], f32, name="lv", tag="lv")
                    nc.scalar.activation(lv, mv[:, 1:2], AF.Sqrt, bias=eps_t[:, 0:1])
                    rstd = stat.tile([128, 1], f32, name="rstd", tag="rstd")
                    nc.vector.reciprocal_approx_fast(rstd, lv)
                    z_t = zp.tile([128, IN], f32r, name="z_t", tag="z_t")
                    nc.vector.tensor_scalar(z_t, x_t, mv[:, 0:1], rstd, op0=OP.subtract, op1=OP.mult)
                    tp = tpp.tile([128, 1024], f32r, name="tp", tag="tp")
                    for c in range(8):
                        nc.tensor.transpose(tp[:, 128 * c : 128 * (c + 1)], z_t[:, 128 * c : 128 * (c + 1)], ident)
                    nc.any.tensor_copy(zT[:, :, 128 * tt : 128 * (tt + 1)], tp.rearrange("p (c w) -> p c w", c=8))
                return zT

            def emit_proj(blk, zT):
                # q/k projections (transposed layout), per head-pair j.
                # PSUM->SBUF copy + bias-add fused on the scalar engine.
                for wsb, cbs, dst in ((wq_sb, cb_q, qT), (wk_sb, cb_k, kT)):
                    for j in range(2):
                        ps = projp.tile([128, BLK], f32, name="proj", tag="proj")
                        for c in range(8):
                            nc.tensor.matmul(ps, wsb[c][:, 128 * j : 128 * (j + 1)], zT[:, c, :],
                                             start=(c == 0), stop=(c == 7))
                        nc.scalar.activation(dst[j][:, BLK * blk : BLK * (blk + 1)], ps,
                                             AF.Identity, bias=cbs[j])
                # v projection (normal layout) per 128-token tile
                for tt in range(4):
                    psv = vpp.tile([128, FH], f32, name="psv", tag="psv")
                    for c in range(8):
                        nc.tensor.matmul(psv, zT[:, c, 128 * tt : 128 * (tt + 1)], wv_sb[c],
                                         start=(c == 0), stop=False)
                    nc.tensor.matmul(psv, ones_r, cv_row, start=False, stop=True)
                    vt = v_tiles[4 * blk + tt]
                    for hh in range(HG):
                        nc.any.tensor_copy(vt[:, hh, 0:64], psv[:, 64 * hh : 64 * (hh + 1)])

            zT0 = emit_tts(0)
            emit_p0_heavy(p0sb, projp, tpp)
            emit_proj(0, zT0)
            for blk in range(1, NBLK):
                zTb = emit_tts(blk)
                emit_proj(blk, zTb)

        # ---------------- Phases 2-4: attention, out-proj, chunked RS + final LN ----------------
        gout_rep = persist.tile([128, IN], f32, name="gout_rep", tag="gout_rep")
        nc.gpsimd.dma_start(gout_rep, outg_d.ap().unsqueeze(0).to_broadcast([128, IN]))
        bout_rep = persist.tile([128, IN], f32, name="bout_rep", tag="bout_rep")
        nc.gpsimd.dma_start(bout_rep, outb_d.ap().unsqueeze(0).to_broadcast([128, IN]))
        with tc.tile_pool(name="wtp", bufs=2) as wtp, \
             tc.tile_pool(name="oddp", bufs=2) as oddp, \
             tc.tile_pool(name="rcpp", bufs=2) as rcpp, \
             tc.tile_pool(name="expnp", bufs=2) as expnp, \
             tc.tile_pool(name="ysb", bufs=3) as ysbp, \
             tc.tile_pool(name="fin", bufs=2) as fin, \
             tc.tile_pool(name="s0p", bufs=3, space="PSUM") as s0p, \
             tc.tile_pool(name="pvp", bufs=2, space="PSUM") as pvp:
            deferred = []
            deferred_fin = []
            fin_steps = []   # previous block's out-proj+RS tiles
            for blk in range(NBLK):
                bsl = slice(BLK * blk, BLK * (blk + 1))
                for pj in range(2):
                    q0 = qT[pj][0:64, bsl]
                    q1 = qT[pj][64:128, bsl]
                    # null-key scores for both heads -> one psum row, one exp
                    expn = expnp.tile([1, 2 * BLK], bf16, name="expn", tag="expn")
                    ps_nl = s0p.tile([1, 2 * BLK], f32, name="ps_nl", tag="ps_s")
                    nc.tensor.matmul(ps_nl[0:1, 0:BLK], knull2[0:64, :], q0, start=True, stop=True)
                    nc.tensor.matmul(ps_nl[0:1, BLK : 2 * BLK], knull2[64:128, :], q1, start=True,
                                     stop=True, tile_position=(64, 0))
                    nc.scalar.activation(expn, ps_nl, AF.Exp, scale=SCALE)
                    # scores -> exp -> PV, pipelined per key tile; both heads share one
                    # [128,1024] scores psum + one exp op (h0 cols 0:512, h1 cols 512:1024).
                    # PV trails a few key tiles behind so PE never head-of-line blocks on exp.
                    ps_pv0 = pvp.tile([65, BLK], f32, name="ps_pv0", tag="ps_pv")
                    ps_pv1 = pvp.tile([65, BLK], f32, name="ps_pv1", tag="ps_pv")

                    def pv_step(kt, wt):
                        lv0 = cv_ext[:, 0:65] if kt == 16 else v_tiles[kt][:, 2 * pj, :]
                        lv1 = cv_ext[:, 0:65] if kt == 16 else v_tiles[kt][:, 2 * pj + 1, :]
                        nc.tensor.matmul(ps_pv0, lv0, wt[:, 0:BLK], start=(kt == 0), stop=False)
                        nc.tensor.matmul(ps_pv1, lv1, wt[:, BLK : 2 * BLK], start=(kt == 0), stop=False)

                    pending = []
                    for kt in range(KT):
                        # do_norm pops before the first pv_step of this pair so the
                        # ps_pv slots are released by its DVE mults in time.
                        if kt == 6 and deferred:
                            deferred.pop(0)()
                        if pj == 0 and kt in (7, 8, 9, 10, 11, 12, 13, 14) and fin_steps:
                            fin_steps.pop(0)()
                        if kt == 14 and pj == 1 and len(deferred_fin) >= 2:
                            deferred_fin.pop(0)()
                        ps_s = s0p.tile([128, 2 * BLK], f32, name="ps_s", tag="ps_s")
                        wt = wtp.tile([128, 2 * BLK], bf16, name="wt", tag="wt", bufs=9)
                        l0 = ck2[0:64, :] if kt == 16 else kT[pj][0:64, 128 * kt : 128 * (kt + 1)]
                        l1 = ck2[64:128, :] if kt == 16 else kT[pj][64:128, 128 * kt : 128 * (kt + 1)]
                        nc.tensor.matmul(ps_s[:, 0:BLK], l0, q0, start=True, stop=True)
                        nc.tensor.matmul(ps_s[:, BLK : 2 * BLK], l1, q1, start=True, stop=True,
                                         tile_position=(64, 0))
                        if len(pending) >= 7:
                            pv_step(*pending.pop(0))
                        nc.scalar.activation(wt, ps_s, AF.Exp, scale=SCALE)
                        pending.append((kt, wt))
                    for args in pending:
                        pv_step(*args)
                    nc.tensor.matmul(ps_pv0, nullv2[0:1, :], expn[0:1, 0:BLK], start=False, stop=True)
                    nc.tensor.matmul(ps_pv1, nullv2[0:1, :], expn[0:1, BLK : 2 * BLK], start=False, stop=True)

                    # normalize: attnT = pv[0:64] * broadcast(1/denominator).  The
                    # denominator row goes PSUM p64 -> SBUF p64 (ACT) -> DRAM -> SBUF
                    # [64, BLK] partition-broadcast (DMA), reciprocal + multiply on DVE
                    # (no PE involvement at all).
                    rcps = []
                    for h, ps_pv in ((0, ps_pv0), (1, ps_pv1)):
                        didx = (blk * 2 + pj) * 2 + h
                        den64 = rcpp.tile([65, BLK], f32, name="den64", tag="den64")
                        nc.scalar.copy(den64[64:65, :], ps_pv[64:65, :])
                        nc.sync.dma_start(dden_d.ap()[didx : didx + 1, :], den64[64:65, :])
                        den_b = rcpp.tile([64, BLK], f32, name="den_b", tag="den_b")
                        nc.sync.dma_start(den_b, dden_d.ap()[didx, :].unsqueeze(0).to_broadcast([64, BLK]))
                        rb = rcpp.tile([64, BLK], f32, name="rb", tag="rb")
                        nc.vector.reciprocal_approx_fast(rb, den_b)
                        rcps.append(rb)

                    def do_norm(pj=pj, bsl=bsl, pvs=(ps_pv0, ps_pv1), rcps=tuple(rcps)):
                        for h, (ps_pv, rb) in enumerate(zip(pvs, rcps)):
                            if h == 0:
                                nc.vector.tensor_tensor(attnT[pj][0:64, bsl], ps_pv[0:64, :], rb, op=OP.mult)
                            else:
                                tmp = oddp.tile([64, BLK], f32r, name="odd", tag="odd")
                                nc.vector.tensor_tensor(tmp, ps_pv[0:64, :], rb, op=OP.mult)
                                nc.sync.dma_start(attnT[pj][64:128, bsl], tmp)

                    deferred.append(do_norm)
                    if pj == 0 and fin_steps:
                        # previous block's remaining out-proj tiles
                        while fin_steps:
                            fin_steps.pop(0)()

                # out-projection + per-tile RS for this block, deferred into the
                # next block's pair-0 key loop so the block boundary never stalls
                # on the pair-1 normalize chain
                def make_ostep(blk=blk, tt4=0):
                    def ostep(tt4=tt4, blk=blk):
                        tt = 4 * blk + tt4
                        y_sb = ysbp.tile([128, IN], bf16, name="y_sb", tag="y_sb")
                        for nh in range(2):
                            ps_y = s0p.tile([128, 512], f32, name="ps_y", tag="ps_s")
                            for c in range(2):
                                nc.tensor.matmul(ps_y, attnT[c][:, 128 * tt : 128 * (tt + 1)],
                                                 wout_sb[c][:, 512 * nh : 512 * (nh + 1)],
                                                 start=(c == 0), stop=(c == 1))
                            nc.vector.tensor_copy(y_sb[:, 512 * nh : 512 * (nh + 1)], ps_y)
                        nc.sync.dma_start(ypart_d[tt].ap(), y_sb)
                        # per-tile ReduceScatter: rank r receives rows [32r, 32r+32)
                        nc.gpsimd.collective_compute(
                            "ReduceScatter",
                            OP.add,
                            replica_groups=[[0, 1, 2, 3], [4, 5, 6, 7]],
                            ins=[ypart_d[tt].ap()],
                            outs=[yred_d[tt].ap()],
                        )
                        if tt4 == 3:
                            deferred_fin.append(make_final_ln(blk))
                    return ostep

                fin_steps = [make_ostep(blk, t4) for t4 in range(4)]

                # final LN per block on the received 4x32 rows -- stats + normalize
                # on the ACT engine (accumulator row-sums, fused scale/bias), quake
                # rsqrt small chain on DVE, gamma/beta application on Pool, so no
                # single engine queue is head-blocked waiting on the collective.
                def make_final_ln(blk=blk):
                  def final_ln(blk=blk):
                    yr_b = fin.tile([128, IN], bf16, name="yr_b", tag="yr_b")
                    for tt4 in range(4):
                        nc.gpsimd.dma_start(yr_b[32 * tt4 : 32 * (tt4 + 1), :],
                                            yred_d[4 * blk + tt4].ap())
                    yr = fin.tile([128, IN], f32, name="yr", tag="yr")
                    rsum = stat.tile([128, 1], f32, name="rsum", tag="rsum")
                    nc.scalar.activation(yr, yr_b, AF.Identity, accum_out=rsum)
                    ysq = fin.tile([128, IN], f32, name="ysq", tag="ysq")
                    rsumsq = stat.tile([128, 1], f32, name="rsumsq", tag="rsumsq")
                    nc.scalar.activation(ysq, yr_b, AF.Square, accum_out=rsumsq)
                    mean = stat.tile([128, 1], f32, name="mean", tag="mean")
                    nc.vector.tensor_scalar_mul(mean, rsum, 1.0 / IN)
                    # vpe = E[y^2] - mean^2 + eps
                    negm2 = stat.tile([128, 1], f32, name="negm2", tag="negm2")
                    nc.vector.scalar_tensor_tensor(negm2, mean, -1.0, mean, op0=OP.mult, op1=OP.mult)
                    varr = stat.tile([128, 1], f32, name="varr", tag="varr")
                    nc.vector.scalar_tensor_tensor(varr, rsumsq, 1.0 / IN, negm2, op0=OP.mult, op1=OP.add)
                    vpe = stat.tile([128, 1], f32, name="vpe", tag="vpe")
                    nc.vector.tensor_scalar_add(vpe, varr, EPS)
                    # rstd = (var+eps)^-1/2 on DVE only (quake seed + 2 Newton
                    # steps) -- keeps the ACT engine's exp table resident.
                    rstd = stat.tile([128, 1], f32, name="rstdq", tag="rstdq")
                    tq = stat.tile([128, 1], f32, name="tq", tag="tq")
                    nc.vector.tensor_scalar(rstd.bitcast(i32), vpe.bitcast(i32), 1, -1,
                                            op0=OP.logical_shift_right, op1=OP.bitwise_xor)
                    nc.vector.tensor_scalar_add(rstd.bitcast(i32), rstd.bitcast(i32), 0x5F3759E0)
                    for _ in range(2):
                        nc.vector.tensor_tensor(tq, rstd, rstd, op=OP.mult)
                        nc.vector.tensor_tensor(tq, tq, vpe, op=OP.mult)
                        nc.vector.tensor_scalar(tq, tq, -0.5, 1.5, op0=OP.mult, op1=OP.add)
                        nc.vector.tensor_tensor(rstd, rstd, tq, op=OP.mult)
                    negmur = stat.tile([128, 1], f32, name="negmur", tag="negmur")
                    nc.vector.scalar_tensor_tensor(negmur, mean, -1.0, rstd, op0=OP.mult, op1=OP.mult)
                    # zf = yr*rstd - mean*rstd (one fused ACT op), then *g +b on Pool
                    zf = fin.tile([128, IN], f32, name="zf", tag="zf")
                    nc.scalar.activation(zf, yr, AF.Identity, scale=rstd[:, 0:1], bias=negmur[:, 0:1])
                    nc.gpsimd.tensor_tensor(zf, zf, gout_rep, op=OP.mult)
                    of = fin.tile([128, IN], f32, name="of", tag="of")
                    nc.gpsimd.tensor_tensor(of, zf, bout_rep, op=OP.add)
                    nc.gpsimd.dma_start(y_out_d.ap()[128 * blk : 128 * (blk + 1), :], of)
                  return final_ln

            # tail: last block's normalize, out-proj + RS, remaining final LNs
            while deferred:
                deferred.pop(0)()
            while fin_steps:
                fin_steps.pop(0)()
            while deferred_fin:
                deferred_fin.pop(0)()


def shard_inputs(inputs):
    """Split full inputs into 8 per-core input maps."""
    x = np.ascontiguousarray(np.asarray(inputs["x"], dtype=np.float32))
    c_emb = np.ascontiguousarray(np.asarray(inputs["c_emb"], dtype=np.float32))
    W_q = np.asarray(inputs["W_q"], np.float32).reshape(IN, H, D)
    W_kv = np.asarray(inputs["W_kv"], np.float32).reshape(IN, 2, H, D)
    W_out = np.asarray(inputs["W_out"], np.float32).reshape(H, D, IN)
    common = {
        "const_ident": np.eye(128, dtype=np.float32),
        "wctx": np.ascontiguousarray(np.asarray(inputs["W_ctx"], np.float32)),
        "nullkv": np.ascontiguousarray(np.asarray(inputs["null_kv"], np.float32)),
        "ln_g": np.ascontiguousarray(np.asarray(inputs["ln_g"], np.float32)),
        "ln_b": np.ascontiguousarray(np.asarray(inputs["ln_b"], np.float32)),
        "ctx_g": np.ascontiguousarray(np.asarray(inputs["ctx_ln_g"], np.float32)),
        "ctx_b": np.ascontiguousarray(np.asarray(inputs["ctx_ln_b"], np.float32)),
        "b_ctx": np.ascontiguousarray(np.asarray(inputs["b_ctx"], np.float32)),
        "out_g": np.ascontiguousarray(np.asarray(inputs["out_ln_g"], np.float32)),
        "out_b": np.ascontiguousarray(np.asarray(inputs["out_ln_b"], np.float32)),
    }
    in_maps = []
    for c in range(NCORES):
        b, g = c // 4, c % 4
        hs = slice(HG * g, HG * (g + 1))
        in_maps.append({
            "x_loc": x[b],
            "cemb_loc": c_emb[b],
            "wq_loc": np.ascontiguousarray(W_q[:, hs].reshape(IN, FH)),
            "wk_loc": np.ascontiguousarray(W_kv[:, 0, hs].reshape(IN, FH)),
            "wv_loc": np.ascontiguousarray(W_kv[:, 1, hs].reshape(IN, FH)),
            "wout_loc": np.ascontiguousarray(W_out[hs].reshape(FH, IN)),
            **common,
        })
    return in_maps


def unshard(results):
    out = np.empty((B, N, IN), np.float32)
    for c in range(NCORES):
        b, r = c // 4, c % 4
        y = results[c]["y_out"]
        for blk in range(NBLK):
            for tt4 in range(4):
                t0 = BLK * blk + 128 * tt4 + 32 * r
                y0 = 128 * blk + 32 * tt4
                out[b, t0 : t0 + 32, :] = y[y0 : y0 + 32]
    return out


_CACHE = {}


def kernel(**inputs) -> np.ndarray:
    from concourse.bass_utils import run_bass_kernel_spmd

    if "nc" not in _CACHE:
        _CACHE["nc"] = build_program()
    nc = _CACHE["nc"]
    in_maps = shard_inputs(inputs)
    res = run_bass_kernel_spmd(nc, in_maps, list(range(NCORES))).results
    return unshard(res)


if __name__ == "__main__":
    nc = build_program()
    print("program built OK;",
          sum(1 for _ in nc.inst_map), "instructions")


# revision 23
# speedup vs baseline: 1.1863x; 1.1863x over previous
"""Trainium2 Bass kernel for nn_MultiHeadAttention_81999515616076.

Reference computation (per batch b):
    xn = LN(x)                                    [N, IN]
    q  = xn @ W_q   -> [N, H, D]
    k,v= xn @ W_kv  -> [N, H, D] each
    ckv= LN(c_emb) @ W_ctx + b_ctx -> ck, cv      [M, D] (shared across heads)
    keys per head = [self keys (N)] + [null key] + [ctx keys (M)]  (2177 total)
    out = softmax(q.k / sqrt(D)) @ values         [N, H, D]
    y  = LN(out.reshape(N, H*D) @ W_out)          [N, IN]

Sharding (8 cores): core c -> batch b = c//4, head group g = c%4 (heads 4g..4g+3).
Per-core: LN+transpose of x, fp32r projections, flash-style attention for its 4
heads (scores computed transposed: [keys, tokens]; softmax denominator via a
ones-column in the PV matmul; no max subtraction -- scores are bounded ~N(0,0.4)),
out-projection partials, per-128-token-tile ReduceScatter(add) over the 4 cores
of each batch, and final LN (entirely on the Pool engine) on the received rows.
Host only slices inputs / concatenates outputs.

Perf notes vs the original baseline (427964 ns):
  - exp weights + PV operands in bf16 (halves ACT write traffic + SBUF reads).
  - softmax normalize via DMA-broadcast of the reciprocal row (no PE broadcast
    matmuls, no PSUM->SBUF casts on DVE).
  - ReduceScatter chunked per 128-token tile so the tail collective is 256KB.
  - final LN on Pool (scalar_tensor_tensor with accum stats), keeping the DVE
    and ACT queues free of collective-dependent work.
  - startup: x DMAs own the sync queue; constants via memset/gpsimd queue.
"""

import sys

sys.path.insert(0, "/opt/trn_rl_repo")

import numpy as np

import concourse.bacc as bacc
import concourse.tile as tile
import concourse.mybir as mybir

B, N, IN = 2, 2048, 1024
H, D = 16, 64
CTX_DIM, M_CTX = 768, 128
NCORES = 8
HG = 4               # heads per core
FH = HG * D          # 256 local head-feats
BLK = 512            # token block
NBLK = N // BLK      # 4
NT = N // 128        # 16 128-token tiles
KT = 17              # 16 self key tiles + 1 ctx key tile (null key handled separately)
SCALE = D ** -0.5    # 0.125
EPS = 1e-5

f32 = mybir.dt.float32
f32r = mybir.dt.float32r
bf16 = mybir.dt.bfloat16
i32 = mybir.dt.int32
AF = mybir.ActivationFunctionType
OP = mybir.AluOpType


def build_program():
    nc = bacc.Bacc("TRN2", target_bir_lowering=False, debug=False, num_devices=NCORES)

    # ---- per-core DRAM tensors (values sharded by host) ----
    x_d = nc.dram_tensor("x_loc", [N, IN], f32, kind="ExternalInput")
    wq_d = nc.dram_tensor("wq_loc", [IN, FH], f32, kind="ExternalInput")
    wk_d = nc.dram_tensor("wk_loc", [IN, FH], f32, kind="ExternalInput")
    wv_d = nc.dram_tensor("wv_loc", [IN, FH], f32, kind="ExternalInput")
    wout_d = nc.dram_tensor("wout_loc", [FH, IN], f32, kind="ExternalInput")
    wctx_d = nc.dram_tensor("wctx", [CTX_DIM, 2 * D], f32, kind="ExternalInput")
    cemb_d = nc.dram_tensor("cemb_loc", [M_CTX, CTX_DIM], f32, kind="ExternalInput")
    nullkv_d = nc.dram_tensor("nullkv", [2, D], f32, kind="ExternalInput")
    lng_d = nc.dram_tensor("ln_g", [IN], f32, kind="ExternalInput")
    lnb_d = nc.dram_tensor("ln_b", [IN], f32, kind="ExternalInput")
    ctxg_d = nc.dram_tensor("ctx_g", [CTX_DIM], f32, kind="ExternalInput")
    ctxb_d = nc.dram_tensor("ctx_b", [CTX_DIM], f32, kind="ExternalInput")
    bctx_d = nc.dram_tensor("b_ctx", [2 * D], f32, kind="ExternalInput")
    outg_d = nc.dram_tensor("out_g", [IN], f32, kind="ExternalInput")
    outb_d = nc.dram_tensor("out_b", [IN], f32, kind="ExternalInput")
    ident_d = nc.dram_tensor("const_ident", [128, 128], f32, kind="ExternalInput")
    y_out_d = nc.dram_tensor("y_out", [BLK, IN], f32, kind="ExternalOutput")
    # internal DRAM for the collective (per 128-token tile to minimize the tail).
    # Partials travel in bf16 to nearly halve collective time.
    ypart_d = [nc.dram_tensor(f"y_partial{b}", [BLK, IN], bf16) for b in range(NBLK)]
    yred_d = [nc.dram_tensor(f"y_red{b}", [128, IN], bf16) for b in range(NBLK)]
    # bounce buffer for the softmax-denominator partition broadcast
    dden_d = nc.dram_tensor("dden", [16, BLK], f32)
    warm_in_d = nc.dram_tensor("warm_in", [128, 8], bf16)
    warm_out_d = nc.dram_tensor("warm_out", [32, 8], bf16)

    with tile.TileContext(nc) as tc:
        _emit(nc, tc, locals())
    nc.compile()
    return nc


def _emit(nc, tc, t):
    from contextlib import ExitStack

    x_d, cemb_d = t["x_d"], t["cemb_d"]
    wq_d, wk_d, wv_d, wout_d, wctx_d = t["wq_d"], t["wk_d"], t["wv_d"], t["wout_d"], t["wctx_d"]
    nullkv_d, bctx_d = t["nullkv_d"], t["bctx_d"]
    lng_d, lnb_d, ctxg_d, ctxb_d = t["lng_d"], t["lnb_d"], t["ctxg_d"], t["ctxb_d"]
    outg_d, outb_d = t["outg_d"], t["outb_d"]
    y_out_d, ypart_d, yred_d = t["y_out_d"], t["ypart_d"], t["yred_d"]
    ident_d, dden_d = t["ident_d"], t["dden_d"]

    with ExitStack() as ctx:
        persist = ctx.enter_context(tc.tile_pool(name="persist", bufs=1))
        stat = ctx.enter_context(tc.tile_pool(name="stat", bufs=4))

        # ---------------- Phase 0: constants & first x tiles ----------------
        # Tiny ReduceScatter to absorb the ~2x cold-start cost of the first
        # collective while phase 1 runs (result unused).
        nc.gpsimd.collective_compute(
            "ReduceScatter",
            OP.add,
            replica_groups=[[0, 1, 2, 3], [4, 5, 6, 7]],
            ins=[t["warm_in_d"].ap()],
            outs=[t["warm_out_d"].ap()],
        )
        # constants that need no DMA: memsets (cheap, no queue dependencies)
        eps_t = persist.tile([128, 1], f32, name="eps", tag="eps")
        nc.vector.memset(eps_t, EPS)
        ones_f = persist.tile([1, 128], f32, name="ones_f", tag="ones_f")
        nc.vector.memset(ones_f, 1.0)
        ones_r = ones_f.bitcast(f32r)

        # small const DMAs on the gpsimd queue (sync queue is reserved for x)
        ident = persist.tile([128, 128], f32r, name="ident", tag="ident")
        nc.gpsimd.dma_start(ident, ident_d.ap().bitcast(f32r))
        g_sb = persist.tile([128, 8], f32, name="g_sb", tag="g_sb")
        nc.gpsimd.dma_start(g_sb, lng_d.ap().rearrange("(c p) -> p c", p=128))
        lnb_sb = persist.tile([128, 8], f32r, name="lnb_sb", tag="lnb_sb")
        nc.gpsimd.dma_start(lnb_sb, lnb_d.ap().rearrange("(c p) -> p c", p=128).bitcast(f32r))
        ctxg_sb = persist.tile([128, 6], f32, name="ctxg_sb", tag="ctxg_sb")
        nc.gpsimd.dma_start(ctxg_sb, ctxg_d.ap().rearrange("(c p) -> p c", p=128))
        ctxb_sb = persist.tile([128, 6], f32r, name="ctxb_sb", tag="ctxb_sb")
        nc.gpsimd.dma_start(ctxb_sb, ctxb_d.ap().rearrange("(c p) -> p c", p=128).bitcast(f32r))

        # null key/value: knull2 rows 0:64 and 64:128 both = null_k (for the two
        # row-packed head positions); nullv2 = [null_v | 1] in bf16 (PV operand).
        knull_st = stat.tile([64, 1], f32, name="knull_st", tag="knull_st")
        nc.gpsimd.dma_start(knull_st, nullkv_d.ap()[0:1, :].rearrange("a b -> b a"))
        knull2 = persist.tile([128, 1], bf16, name="knull2", tag="knull2")
        nc.vector.tensor_copy(knull2[0:64, :], knull_st)
        nc.gpsimd.dma_start(knull2[64:128, :], knull2[0:64, :])
        nullv_f32 = stat.tile([1, 64], f32, name="nullv_f32", tag="nullv_f32")
        nc.gpsimd.dma_start(nullv_f32, nullkv_d.ap()[1:2, :])
        nullv2 = persist.tile([1, 65], bf16, name="nullv2", tag="nullv2")
        nc.vector.tensor_copy(nullv2[0:1, 0:64], nullv_f32)
        nc.vector.memset(nullv2[0:1, 64:65], 1.0)

        # Heavy P0 (weights + context projection), emitted AFTER block-0's LN/transpose
        # chains so the first x tiles hit the DMA queue first.
        wq_sb, wk_sb, wv_sb, wctx_sb, wout_sb = [], [], [], [], []
        cb_q, cb_k = [], []
        cv_row = persist.tile([1, FH], f32r, name="cv_row", tag="cv_row")
        ckvT_sb = persist.tile([128, M_CTX], f32r, name="ckvT", tag="ckvT")
        ck2 = persist.tile([128, M_CTX], bf16, name="ck2", tag="ck2")
        cv_ext = persist.tile([128, 65], bf16, name="cv_ext", tag="cv_ext")

        def emit_p0_heavy(p0sb, psP, psT):
            # weight DMAs: wq/wk on the scalar queue but scheduled after block-0's
            # LN sqrt chain (tile_wait_until); wv/wctx/wout on the gpsimd queue.
            # Per-feature LN gamma folded in on DVE.
            for name, dram, lst, eng, wait in (
                ("wq", wq_d, wq_sb, nc.scalar, 0.012), ("wk", wk_d, wk_sb, nc.scalar, 0.014),
                ("wv", wv_d, wv_sb, nc.gpsimd, 0.0),
            ):
                for c in range(8):
                    w = persist.tile([128, FH], f32r, name=f"{name}{c}", tag=f"{name}{c}")
                    with tc.tile_wait_until(wait, enable=wait > 0):
                        eng.dma_start(w, dram.ap()[128 * c : 128 * (c + 1), :].bitcast(f32r))
                    nc.vector.tensor_scalar_mul(w, w, g_sb[:, c : c + 1])
                    lst.append(w)
            for c in range(6):
                w = persist.tile([128, 2 * D], f32r, name=f"wctx{c}", tag=f"wctx{c}")
                nc.gpsimd.dma_start(w, wctx_d.ap()[128 * c : 128 * (c + 1), :].bitcast(f32r))
                nc.vector.tensor_scalar_mul(w, w, ctxg_sb[:, c : c + 1])
                wctx_sb.append(w)
            for c in range(2):
                wst = p0sb.tile([128, IN], f32, name=f"wout_st{c}", tag="wout_st")
                nc.gpsimd.dma_start(wst, wout_d.ap()[128 * c : 128 * (c + 1), :])
                w = persist.tile([128, IN], bf16, name=f"wout{c}", tag=f"wout{c}")
                nc.vector.tensor_copy(w, wst)
                wout_sb.append(w)
            # LN-beta folded biases: cb[j] = (ln_b @ W')[128j:128j+128] as [128,1]
            for wsb, lst in ((wq_sb, cb_q), (wk_sb, cb_k)):
                for j in range(2):
                    ps = psP.tile([128, 1], f32, name="p0bias", tag="proj")
                    for c in range(8):
                        nc.tensor.matmul(ps, wsb[c][:, 128 * j : 128 * (j + 1)].bitcast(f32),
                                         lnb_sb[:, c : c + 1].bitcast(f32), start=(c == 0), stop=(c == 7))
                    cb = persist.tile([128, 1], f32, name=f"cb{len(lst)}_{id(wsb) % 97}", tag=f"cb{len(cb_q)}_{len(cb_k)}")
                    nc.vector.tensor_copy(cb, ps)
                    lst.append(cb)
            # v bias as a row [1, FH] (added via a K=1 ones matmul)
            psc = psP.tile([1, FH], f32, name="p0cv", tag="proj")
            for c in range(8):
                nc.tensor.matmul(psc, lnb_sb[:, c : c + 1], wv_sb[c], start=(c == 0), stop=(c == 7))
            nc.vector.tensor_copy(cv_row, psc)
            # ---- context projection: ckv^T = W_ctx'.T @ LN(c_emb).T + bias ----
            cemb_sb = p0sb.tile([128, CTX_DIM], f32, name="cemb", tag="cemb")
            nc.gpsimd.dma_start(cemb_sb, cemb_d.ap())
            stc = stat.tile([128, 3, 6], f32, name="stc", tag="stc")
            for i in range(3):
                nc.vector.bn_stats(stc[:, i, :], cemb_sb[:, 256 * i : 256 * (i + 1)])
            mvc = stat.tile([128, 2], f32, name="mvc", tag="mvc")
            nc.vector.bn_aggr(mvc, stc)
            rstd_c = stat.tile([128, 1], f32, name="rstd_c", tag="rstd_c")
            lnv = stat.tile([128, 1], f32, name="lnv", tag="lnv")
            nc.scalar.activation(lnv, mvc[:, 1:2], AF.Sqrt, bias=eps_t[:, 0:1])
            nc.vector.reciprocal_approx_fast(rstd_c, lnv)
            zc = p0sb.tile([128, CTX_DIM], f32r, name="zc", tag="zc")
            nc.vector.tensor_scalar(zc, cemb_sb, mvc[:, 0:1], rstd_c, op0=OP.subtract, op1=OP.mult)
            tpc = psT.tile([128, CTX_DIM], f32r, name="tpc", tag="tp")
            for c in range(6):
                nc.tensor.transpose(tpc[:, 128 * c : 128 * (c + 1)], zc[:, 128 * c : 128 * (c + 1)], ident)
            zcT = p0sb.tile([128, 6, 128], f32r, name="zcT", tag="zcT")
            nc.any.tensor_copy(zcT, tpc.rearrange("p (c w) -> p c w", c=6))
            # bias = (ctx_b @ W_ctx')^T + b_ctx
            psb2 = psP.tile([128, 1], f32, name="p0bias2", tag="proj")
            for c in range(6):
                nc.tensor.matmul(psb2, wctx_sb[c].bitcast(f32), ctxb_sb[:, c : c + 1].bitcast(f32),
                                 start=(c == 0), stop=(c == 5))
            bctx_sb = stat.tile([128, 1], f32, name="bctx_sb", tag="bctx_sb")
            nc.gpsimd.dma_start(bctx_sb, bctx_d.ap().rearrange("(a p) -> p a", p=128))
            ckv_bias = stat.tile([128, 1], f32, name="ckv_bias", tag="ckv_bias")
            nc.vector.tensor_tensor(ckv_bias, psb2, bctx_sb, op=OP.add)
            psk = psP.tile([128, M_CTX], f32, name="psk", tag="proj")
            for c in range(6):
                nc.tensor.matmul(psk, wctx_sb[c], zcT[:, c, :], start=(c == 0), stop=(c == 5))
            nc.scalar.activation(ckvT_sb, psk, AF.Identity, bias=ckv_bias)
            # ck duplicated into both row-halves (for 2-head row packing),
            # converted to bf16 lane-aligned then row-shifted via DMA
            nc.vector.tensor_copy(ck2[0:64, :], ckvT_sb[0:64, :])
            nc.sync.dma_start(ck2[64:128, :], ck2[0:64, :])
            # cv in normal layout [M_CTX, 64] with a ones column -> [128, 65] bf16
            cvT_tmp = p0sb.tile([64, M_CTX], f32r, name="cvT_tmp", tag="cvT_tmp")
            nc.sync.dma_start(cvT_tmp, ckvT_sb[64:128, :])
            ps_cv = psT.tile([128, 64], f32r, name="ps_cv", tag="tp")
            nc.tensor.transpose(ps_cv, cvT_tmp, ident[0:64, 0:64])
            nc.any.tensor_copy(cv_ext[:, 0:64], ps_cv)
            nc.vector.memset(cv_ext[:, 64:65], 1.0)

        # ---------------- persistent activation tensors ----------------
        qT = [persist.tile([128, N], bf16, name=f"qT{j}", tag=f"qT{j}") for j in range(2)]
        kT = [persist.tile([128, N], bf16, name=f"kT{j}", tag=f"kT{j}") for j in range(2)]
        attnT = [persist.tile([128, N], bf16, name=f"attnT{j}", tag=f"attnT{j}") for j in range(2)]
        v_tiles = []
        for i in range(16):
            vt = persist.tile([128, HG, 65], bf16, name=f"v{i}", tag=f"v{i}")
            nc.gpsimd.memset(vt[:, :, 64:65], 1.0)
            v_tiles.append(vt)

        # ---------------- Phase 1: LN(x), transpose, q/k/v projections ----------------
        with tc.tile_pool(name="xp", bufs=3) as xp, \
             tc.tile_pool(name="zp", bufs=2) as zp, \
             tc.tile_pool(name="ztp", bufs=2) as ztp, \
             tc.tile_pool(name="p0sb", bufs=2) as p0sb, \
             tc.tile_pool(name="tpp", bufs=2, space="PSUM") as tpp, \
             tc.tile_pool(name="projp", bufs=2, space="PSUM") as projp, \
             tc.tile_pool(name="vpp", bufs=2, space="PSUM") as vpp:

            def emit_tts(blk):
                zT = ztp.tile([128, 8, BLK], f32r, name="zT", tag="zT")
                for tt in range(4):
                    t0 = BLK * blk + 128 * tt
                    x_t = xp.tile([128, IN], f32, name="x_t", tag="x_t")
                    nc.sync.dma_start(x_t, x_d.ap()[t0 : t0 + 128, :])
                    st = stat.tile([128, 2, 6], f32, name="st", tag="st")
                    nc.vector.bn_stats(st[:, 0, :], x_t[:, 0:512])
                    nc.vector.bn_stats(st[:, 1, :], x_t[:, 512:1024])
                    mv = stat.tile([128, 2], f32, name="mv", tag="mv")
                    nc.vector.bn_aggr(mv, st)
                    lv = stat.tile([128, 1], f32, name="lv", tag="lv")
                    nc.scalar.activation(lv, mv[:, 1:2], AF.Sqrt, bias=eps_t[:, 0:1])
                    rstd = stat.tile([128, 1], f32, name="rstd", tag="rstd")
                    nc.vector.reciprocal_approx_fast(rstd, lv)
                    z_t = zp.tile([128, IN], f32r, name="z_t", tag="z_t")
                    nc.vector.tensor_scalar(z_t, x_t, mv[:, 0:1], rstd, op0=OP.subtract, op1=OP.mult)
                    tp = tpp.tile([128, 1024], f32r, name="tp", tag="tp")
                    for c in range(8):
                        nc.tensor.transpose(tp[:, 128 * c : 128 * (c + 1)], z_t[:, 128 * c : 128 * (c + 1)], ident)
                    nc.vector.tensor_copy(zT[:, :, 128 * tt : 128 * (tt + 1)], tp.rearrange("p (c w) -> p c w", c=8))
                return zT

            def emit_proj(blk, zT):
                # q/k projections (transposed layout), per head-pair j.
                # PSUM->SBUF copy + bias-add fused on the scalar engine.
                for wsb, cbs, dst in ((wq_sb, cb_q, qT), (wk_sb, cb_k, kT)):
                    for j in range(2):
                        ps = projp.tile([128, BLK], f32, name="proj", tag="proj")
                        for c in range(8):
                            nc.tensor.matmul(ps, wsb[c][:, 128 * j : 128 * (j + 1)], zT[:, c, :],
                                             start=(c == 0), stop=(c == 7))
                        nc.scalar.activation(dst[j][:, BLK * blk : BLK * (blk + 1)], ps,
                                             AF.Identity, bias=cbs[j])
                # v projection (normal layout) per 128-token tile
                for tt in range(4):
                    psv = vpp.tile([128, FH], f32, name="psv", tag="psv")
                    for c in range(8):
                        nc.tensor.matmul(psv, zT[:, c, 128 * tt : 128 * (tt + 1)], wv_sb[c],
                                         start=(c == 0), stop=False)
                    nc.tensor.matmul(psv, ones_r, cv_row, start=False, stop=True)
                    vt = v_tiles[4 * blk + tt]
                    for hh in range(HG):
                        nc.scalar.copy(vt[:, hh, 0:64], psv[:, 64 * hh : 64 * (hh + 1)])

            zT0 = emit_tts(0)
            emit_p0_heavy(p0sb, projp, tpp)
            emit_proj(0, zT0)
            for blk in range(1, NBLK):
                zTb = emit_tts(blk)
                emit_proj(blk, zTb)

        # ---------------- Phases 2-4: attention, out-proj, chunked RS + final LN ----------------
        gout_rep = persist.tile([128, IN], f32, name="gout_rep", tag="gout_rep")
        nc.gpsimd.dma_start(gout_rep, outg_d.ap().unsqueeze(0).to_broadcast([128, IN]))
        bout_rep = persist.tile([128, IN], f32, name="bout_rep", tag="bout_rep")
        nc.gpsimd.dma_start(bout_rep, outb_d.ap().unsqueeze(0).to_broadcast([128, IN]))
        with tc.tile_pool(name="wtp", bufs=2) as wtp, \
             tc.tile_pool(name="oddp", bufs=2) as oddp, \
             tc.tile_pool(name="rcpp", bufs=2) as rcpp, \
             tc.tile_pool(name="expnp", bufs=2) as expnp, \
             tc.tile_pool(name="ysb", bufs=3) as ysbp, \
             tc.tile_pool(name="fin", bufs=2) as fin, \
             tc.tile_pool(name="s0p", bufs=3, space="PSUM") as s0p, \
             tc.tile_pool(name="pvp", bufs=2, space="PSUM") as pvp:
            deferred = []
            deferred_fin = []
            fin_steps = []   # previous block's out-proj+RS tiles
            for blk in range(NBLK):
                bsl = slice(BLK * blk, BLK * (blk + 1))
                for pj in range(2):
                    q0 = qT[pj][0:64, bsl]
                    q1 = qT[pj][64:128, bsl]
                    # null-key scores for both heads -> one psum row, one exp
                    expn = expnp.tile([1, 2 * BLK], bf16, name="expn", tag="expn")
                    ps_nl = s0p.tile([1, 2 * BLK], f32, name="ps_nl", tag="ps_s")
                    nc.tensor.matmul(ps_nl[0:1, 0:BLK], knull2[0:64, :], q0, start=True, stop=True)
                    nc.tensor.matmul(ps_nl[0:1, BLK : 2 * BLK], knull2[64:128, :], q1, start=True,
                                     stop=True, tile_position=(64, 0))
                    nc.scalar.activation(expn, ps_nl, AF.Exp, scale=SCALE)
                    # scores -> exp -> PV, pipelined per key tile; both heads share one
                    # [128,1024] scores psum + one exp op (h0 cols 0:512, h1 cols 512:1024).
                    # PV trails a few key tiles behind so PE never head-of-line blocks on exp.
                    ps_pv0 = pvp.tile([65, BLK], f32, name="ps_pv0", tag="ps_pv")
                    ps_pv1 = pvp.tile([65, BLK], f32, name="ps_pv1", tag="ps_pv")

                    def pv_step(kt, wt):
                        lv0 = cv_ext[:, 0:65] if kt == 16 else v_tiles[kt][:, 2 * pj, :]
                        lv1 = cv_ext[:, 0:65] if kt == 16 else v_tiles[kt][:, 2 * pj + 1, :]
                        nc.tensor.matmul(ps_pv0, lv0, wt[:, 0:BLK], start=(kt == 0), stop=False)
                        nc.tensor.matmul(ps_pv1, lv1, wt[:, BLK : 2 * BLK], start=(kt == 0), stop=False)

                    pending = []
                    for kt in range(KT):
                        # do_norm pops before the first pv_step of this pair so the
                        # ps_pv slots are released by its DVE mults in time.
                        if kt == 6 and deferred:
                            deferred.pop(0)()
                        if pj == 0 and kt in (7, 8, 9, 10, 11, 12, 13, 14) and fin_steps:
                            fin_steps.pop(0)()
                        if kt == 14 and pj == 1 and len(deferred_fin) >= 2:
                            deferred_fin.pop(0)()
                        ps_s = s0p.tile([128, 2 * BLK], f32, name="ps_s", tag="ps_s")
                        wt = wtp.tile([128, 2 * BLK], bf16, name="wt", tag="wt", bufs=9)
                        l0 = ck2[0:64, :] if kt == 16 else kT[pj][0:64, 128 * kt : 128 * (kt + 1)]
                        l1 = ck2[64:128, :] if kt == 16 else kT[pj][64:128, 128 * kt : 128 * (kt + 1)]
                        nc.tensor.matmul(ps_s[:, 0:BLK], l0, q0, start=True, stop=True)
                        nc.tensor.matmul(ps_s[:, BLK : 2 * BLK], l1, q1, start=True, stop=True,
                                         tile_position=(64, 0))
                        if len(pending) >= 7:
                            pv_step(*pending.pop(0))
                        nc.scalar.activation(wt, ps_s, AF.Exp, scale=SCALE)
                        pending.append((kt, wt))
                    for args in pending:
                        pv_step(*args)
                    nc.tensor.matmul(ps_pv0, nullv2[0:1, :], expn[0:1, 0:BLK], start=False, stop=True)
                    nc.tensor.matmul(ps_pv1, nullv2[0:1, :], expn[0:1, BLK : 2 * BLK], start=False, stop=True)

                    # normalize: attnT = pv[0:64] * broadcast(1/denominator).  The
                    # denominator row goes PSUM p64 -> SBUF p64 (ACT) -> DRAM -> SBUF
                    # [64, BLK] partition-broadcast (DMA), reciprocal + multiply on DVE
                    # (no PE involvement at all).
                    rcps = []
                    for h, ps_pv in ((0, ps_pv0), (1, ps_pv1)):
                        didx = (blk * 2 + pj) * 2 + h
                        den64 = rcpp.tile([65, BLK], f32, name="den64", tag="den64")
                        nc.scalar.copy(den64[64:65, :], ps_pv[64:65, :])
                        nc.sync.dma_start(dden_d.ap()[didx : didx + 1, :], den64[64:65, :])
                        den_b = rcpp.tile([64, BLK], f32, name="den_b", tag="den_b")
                        nc.sync.dma_start(den_b, dden_d.ap()[didx, :].unsqueeze(0).to_broadcast([64, BLK]))
                        rb = rcpp.tile([64, BLK], f32, name="rb", tag="rb")
                        nc.vector.reciprocal_approx_fast(rb, den_b)
                        rcps.append(rb)

                    def do_norm(pj=pj, bsl=bsl, pvs=(ps_pv0, ps_pv1), rcps=tuple(rcps)):
                        for h, (ps_pv, rb) in enumerate(zip(pvs, rcps)):
                            if h == 0:
                                nc.vector.tensor_tensor(attnT[pj][0:64, bsl], ps_pv[0:64, :], rb, op=OP.mult)
                            else:
                                tmp = oddp.tile([64, BLK], bf16, name="odd", tag="odd")
                                nc.vector.tensor_tensor(tmp, ps_pv[0:64, :], rb, op=OP.mult)
                                nc.sync.dma_start(attnT[pj][64:128, bsl], tmp)

                    deferred.append(do_norm)
                    if pj == 0 and fin_steps:
                        # previous block's remaining out-proj tiles
                        while fin_steps:
                            fin_steps.pop(0)()

                # out-projection + per-tile RS for this block, deferred into the
                # next block's pair-0 key loop so the block boundary never stalls
                # on the pair-1 normalize chain
                def make_ostep(blk=blk, tt4=0):
                    def ostep(tt4=tt4, blk=blk):
                        tt = 4 * blk + tt4
                        y_sb = ysbp.tile([128, IN], bf16, name="y_sb", tag="y_sb")
                        for nh in range(2):
                            ps_y = s0p.tile([128, 512], f32, name="ps_y", tag="ps_s")
                            for c in range(2):
                                nc.tensor.matmul(ps_y, attnT[c][:, 128 * tt : 128 * (tt + 1)],
                                                 wout_sb[c][:, 512 * nh : 512 * (nh + 1)],
                                                 start=(c == 0), stop=(c == 1))
                            nc.vector.tensor_copy(y_sb[:, 512 * nh : 512 * (nh + 1)], ps_y)
                        nc.sync.dma_start(ypart_d[blk].ap()[128 * tt4 : 128 * (tt4 + 1), :], y_sb)
                        if tt4 == 3:
                            # block ReduceScatter: rank r receives rows [128r, 128(r+1))
                            nc.gpsimd.collective_compute(
                                "ReduceScatter",
                                OP.add,
                                replica_groups=[[0, 1, 2, 3], [4, 5, 6, 7]],
                                ins=[ypart_d[blk].ap()],
                                outs=[yred_d[blk].ap()],
                            )
                            deferred_fin.append(make_final_ln(blk))
                    return ostep

                fin_steps = [make_ostep(blk, t4) for t4 in range(4)]

                # final LN per block on the received 4x32 rows -- stats + normalize
                # on the ACT engine (accumulator row-sums, fused scale/bias), quake
                # rsqrt small chain on DVE, gamma/beta application on Pool, so no
                # single engine queue is head-blocked waiting on the collective.
                def make_final_ln(blk=blk):
                  def final_ln(blk=blk):
                    yr_b = fin.tile([128, IN], bf16, name="yr_b", tag="yr_b")
                    nc.gpsimd.dma_start(yr_b, yred_d[blk].ap())
                    yr = fin.tile([128, IN], f32, name="yr", tag="yr")
                    rsum = stat.tile([128, 1], f32, name="rsum", tag="rsum")
                    nc.scalar.activation(yr, yr_b, AF.Identity, accum_out=rsum)
                    ysq = fin.tile([128, IN], f32, name="ysq", tag="ysq")
                    rsumsq = stat.tile([128, 1], f32, name="rsumsq", tag="rsumsq")
                    nc.scalar.activation(ysq, yr_b, AF.Square, accum_out=rsumsq)
                    mean = stat.tile([128, 1], f32, name="mean", tag="mean")
                    nc.vector.tensor_scalar_mul(mean, rsum, 1.0 / IN)
                    # vpe = E[y^2] - mean^2 + eps
                    negm2 = stat.tile([128, 1], f32, name="negm2", tag="negm2")
                    nc.vector.scalar_tensor_tensor(negm2, mean, -1.0, mean, op0=OP.mult, op1=OP.mult)
                    varr = stat.tile([128, 1], f32, name="varr", tag="varr")
                    nc.vector.scalar_tensor_tensor(varr, rsumsq, 1.0 / IN, negm2, op0=OP.mult, op1=OP.add)
                    vpe = stat.tile([128, 1], f32, name="vpe", tag="vpe")
                    nc.vector.tensor_scalar_add(vpe, varr, EPS)
                    # rstd = (var+eps)^-1/2 on DVE only (quake seed + 2 Newton
                    # steps) -- keeps the ACT engine's exp table resident.
                    rstd = stat.tile([128, 1], f32, name="rstdq", tag="rstdq")
                    tq = stat.tile([128, 1], f32, name="tq", tag="tq")
                    nc.vector.tensor_scalar(rstd.bitcast(i32), vpe.bitcast(i32), 1, -1,
                                            op0=OP.logical_shift_right, op1=OP.bitwise_xor)
                    nc.vector.tensor_scalar_add(rstd.bitcast(i32), rstd.bitcast(i32), 0x5F3759E0)
                    for _ in range(2):
                        nc.vector.tensor_tensor(tq, rstd, rstd, op=OP.mult)
                        nc.vector.tensor_tensor(tq, tq, vpe, op=OP.mult)
                        nc.vector.tensor_scalar(tq, tq, -0.5, 1.5, op0=OP.mult, op1=OP.add)
                        nc.vector.tensor_tensor(rstd, rstd, tq, op=OP.mult)
                    negmur = stat.tile([128, 1], f32, name="negmur", tag="negmur")
                    nc.vector.scalar_tensor_tensor(negmur, mean, -1.0, rstd, op0=OP.mult, op1=OP.mult)
                    # zf = yr*rstd - mean*rstd (one fused ACT op), then *g +b on Pool
                    zf = fin.tile([128, IN], f32, name="zf", tag="zf")
                    nc.scalar.activation(zf, yr, AF.Identity, scale=rstd[:, 0:1], bias=negmur[:, 0:1])
                    nc.vector.tensor_tensor(zf, zf, gout_rep, op=OP.mult)
                    of = fin.tile([128, IN], f32, name="of", tag="of")
                    nc.vector.tensor_tensor(of, zf, bout_rep, op=OP.add)
                    nc.gpsimd.dma_start(y_out_d.ap()[128 * blk : 128 * (blk + 1), :], of)
                  return final_ln

            # tail: last block's normalize, out-proj + RS, remaining final LNs
            while deferred:
                deferred.pop(0)()
            while fin_steps:
                fin_steps.pop(0)()
            while deferred_fin:
                deferred_fin.pop(0)()


def shard_inputs(inputs):
    """Split full inputs into 8 per-core input maps."""
    x = np.ascontiguousarray(np.asarray(inputs["x"], dtype=np.float32))
    c_emb = np.ascontiguousarray(np.asarray(inputs["c_emb"], dtype=np.float32))
    W_q = np.asarray(inputs["W_q"], np.float32).reshape(IN, H, D)
    W_kv = np.asarray(inputs["W_kv"], np.float32).reshape(IN, 2, H, D)
    W_out = np.asarray(inputs["W_out"], np.float32).reshape(H, D, IN)
    common = {
        "const_ident": np.eye(128, dtype=np.float32),
        "wctx": np.ascontiguousarray(np.asarray(inputs["W_ctx"], np.float32)),
        "nullkv": np.ascontiguousarray(np.asarray(inputs["null_kv"], np.float32)),
        "ln_g": np.ascontiguousarray(np.asarray(inputs["ln_g"], np.float32)),
        "ln_b": np.ascontiguousarray(np.asarray(inputs["ln_b"], np.float32)),
        "ctx_g": np.ascontiguousarray(np.asarray(inputs["ctx_ln_g"], np.float32)),
        "ctx_b": np.ascontiguousarray(np.asarray(inputs["ctx_ln_b"], np.float32)),
        "b_ctx": np.ascontiguousarray(np.asarray(inputs["b_ctx"], np.float32)),
        "out_g": np.ascontiguousarray(np.asarray(inputs["out_ln_g"], np.float32)),
        "out_b": np.ascontiguousarray(np.asarray(inputs["out_ln_b"], np.float32)),
    }
    in_maps = []
    for c in range(NCORES):
        b, g = c // 4, c % 4
        hs = slice(HG * g, HG * (g + 1))
        in_maps.append({
            "x_loc": x[b],
            "cemb_loc": c_emb[b],
            "wq_loc": np.ascontiguousarray(W_q[:, hs].reshape(IN, FH)),
            "wk_loc": np.ascontiguousarray(W_kv[:, 0, hs].reshape(IN, FH)),
            "wv_loc": np.ascontiguousarray(W_kv[:, 1, hs].reshape(IN, FH)),
            "wout_loc": np.ascontiguousarray(W_out[hs].reshape(FH, IN)),
            **common,
        })
    return in_maps


def unshard(results):
    out = np.empty((B, N, IN), np.float32)
    for c in range(NCORES):
        b, r = c // 4, c % 4
        y = results[c]["y_out"]
        for blk in range(NBLK):
            t0 = BLK * blk + 128 * r
            out[b, t0 : t0 + 128, :] = y[128 * blk : 128 * (blk + 1)]
    return out


_CACHE = {}


def kernel(**inputs) -> np.ndarray:
    from concourse.bass_utils import run_bass_kernel_spmd

    if "nc" not in _CACHE:
        _CACHE["nc"] = build_program()
    nc = _CACHE["nc"]
    in_maps = shard_inputs(inputs)
    res = run_bass_kernel_spmd(nc, in_maps, list(range(NCORES))).results
    return unshard(res)


if __name__ == "__main__":
    nc = build_program()
    print("program built OK;",
          sum(1 for _ in nc.inst_map), "instructions")


# revision 28
# speedup vs baseline: 1.3071x; 1.1019x over previous
"""Trainium2 Bass kernel for nn_MultiHeadAttention_81999515616076.

Reference computation (per batch b):
    xn = LN(x)                                    [N, IN]
    q  = xn @ W_q   -> [N, H, D]
    k,v= xn @ W_kv  -> [N, H, D] each
    ckv= LN(c_emb) @ W_ctx + b_ctx -> ck, cv      [M, D] (shared across heads)
    keys per head = [self keys (N)] + [null key] + [ctx keys (M)]  (2177 total)
    out = softmax(q.k / sqrt(D)) @ values         [N, H, D]
    y  = LN(out.reshape(N, H*D) @ W_out)          [N, IN]

Sharding (8 cores): core c -> batch b = c//4, head group g = c%4 (heads 4g..4g+3).
Per-core: LN+transpose of x, fp32r projections, flash-style attention for its 4
heads (scores computed transposed: [keys, tokens]; softmax denominator via a
ones-column in the PV matmul; no max subtraction -- scores are bounded ~N(0,0.4)),
out-projection partials, per-128-token-tile ReduceScatter(add) over the 4 cores
of each batch, and final LN (entirely on the Pool engine) on the received rows.
Host only slices inputs / concatenates outputs.

Perf notes vs the original baseline (427964 ns):
  - exp weights + PV operands in bf16 (halves ACT write traffic + SBUF reads).
  - softmax normalize via DMA-broadcast of the reciprocal row (no PE broadcast
    matmuls, no PSUM->SBUF casts on DVE).
  - ReduceScatter chunked per 128-token tile so the tail collective is 256KB.
  - final LN on Pool (scalar_tensor_tensor with accum stats), keeping the DVE
    and ACT queues free of collective-dependent work.
  - startup: x DMAs own the sync queue; constants via memset/gpsimd queue.
"""

import sys

sys.path.insert(0, "/opt/trn_rl_repo")

import numpy as np

import concourse.bacc as bacc
import concourse.tile as tile
import concourse.mybir as mybir

B, N, IN = 2, 2048, 1024
H, D = 16, 64
CTX_DIM, M_CTX = 768, 128
NCORES = 8
HG = 4               # heads per core
FH = HG * D          # 256 local head-feats
BLK = 512            # token block
NBLK = N // BLK      # 4
NT = N // 128        # 16 128-token tiles
KT = 17              # 16 self key tiles + 1 ctx key tile (null key handled separately)
SCALE = D ** -0.5    # 0.125
EPS = 1e-5
# Schraudolph-style fast exp emitted as bf16 bits by one DVE op:
#   bf16_bits = round(score*SCALE/ln2*128 + (127*128 - 7.41))
# ~1.8% rms weight error on the offloaded key tiles; offloading 5 of 17
# tiles moves ~30% of the softmax exp load off the saturated ACT engine.
import math
EXP_A = SCALE * 128.0 / math.log(2.0)
EXP_B = 127.0 * 128.0 - 7.41
DVE_KTS = frozenset((2, 5, 8, 11, 14))

f32 = mybir.dt.float32
f32r = mybir.dt.float32r
bf16 = mybir.dt.bfloat16
i32 = mybir.dt.int32
i16 = mybir.dt.int16
AF = mybir.ActivationFunctionType
OP = mybir.AluOpType


def build_program():
    nc = bacc.Bacc("TRN2", target_bir_lowering=False, debug=False, num_devices=NCORES)

    # ---- per-core DRAM tensors (values sharded by host) ----
    x_d = nc.dram_tensor("x_loc", [N, IN], f32, kind="ExternalInput")
    wq_d = nc.dram_tensor("wq_loc", [IN, FH], f32, kind="ExternalInput")
    wk_d = nc.dram_tensor("wk_loc", [IN, FH], f32, kind="ExternalInput")
    wv_d = nc.dram_tensor("wv_loc", [IN, FH], f32, kind="ExternalInput")
    wout_d = nc.dram_tensor("wout_loc", [FH, IN], f32, kind="ExternalInput")
    wctx_d = nc.dram_tensor("wctx", [CTX_DIM, 2 * D], f32, kind="ExternalInput")
    cemb_d = nc.dram_tensor("cemb_loc", [M_CTX, CTX_DIM], f32, kind="ExternalInput")
    nullkv_d = nc.dram_tensor("nullkv", [2, D], f32, kind="ExternalInput")
    lng_d = nc.dram_tensor("ln_g", [IN], f32, kind="ExternalInput")
    lnb_d = nc.dram_tensor("ln_b", [IN], f32, kind="ExternalInput")
    ctxg_d = nc.dram_tensor("ctx_g", [CTX_DIM], f32, kind="ExternalInput")
    ctxb_d = nc.dram_tensor("ctx_b", [CTX_DIM], f32, kind="ExternalInput")
    bctx_d = nc.dram_tensor("b_ctx", [2 * D], f32, kind="ExternalInput")
    outg_d = nc.dram_tensor("out_g", [IN], f32, kind="ExternalInput")
    outb_d = nc.dram_tensor("out_b", [IN], f32, kind="ExternalInput")
    ident_d = nc.dram_tensor("const_ident", [128, 128], f32, kind="ExternalInput")
    y_out_d = nc.dram_tensor("y_out", [BLK, IN], f32, kind="ExternalOutput")
    # internal DRAM for the collective (per 128-token tile to minimize the tail).
    # Partials travel in bf16 to nearly halve collective time.
    ypart_d = [nc.dram_tensor(f"y_partial{b}", [BLK, IN], bf16) for b in range(NBLK)]
    yred_d = [nc.dram_tensor(f"y_red{b}", [128, IN], bf16) for b in range(NBLK)]
    # bounce buffer for the softmax-denominator partition broadcast
    dden_d = nc.dram_tensor("dden", [16, BLK], f32)
    warm_in_d = nc.dram_tensor("warm_in", [128, 8], bf16)
    warm_out_d = nc.dram_tensor("warm_out", [32, 8], bf16)

    with tile.TileContext(nc) as tc:
        _emit(nc, tc, locals())
    nc.compile()
    return nc


def _emit(nc, tc, t):
    from contextlib import ExitStack

    x_d, cemb_d = t["x_d"], t["cemb_d"]
    wq_d, wk_d, wv_d, wout_d, wctx_d = t["wq_d"], t["wk_d"], t["wv_d"], t["wout_d"], t["wctx_d"]
    nullkv_d, bctx_d = t["nullkv_d"], t["bctx_d"]
    lng_d, lnb_d, ctxg_d, ctxb_d = t["lng_d"], t["lnb_d"], t["ctxg_d"], t["ctxb_d"]
    outg_d, outb_d = t["outg_d"], t["outb_d"]
    y_out_d, ypart_d, yred_d = t["y_out_d"], t["ypart_d"], t["yred_d"]
    ident_d, dden_d = t["ident_d"], t["dden_d"]

    with ExitStack() as ctx:
        persist = ctx.enter_context(tc.tile_pool(name="persist", bufs=1))
        stat = ctx.enter_context(tc.tile_pool(name="stat", bufs=4))

        # ---------------- Phase 0: constants & first x tiles ----------------
        # Tiny ReduceScatter to absorb the ~2x cold-start cost of the first
        # collective while phase 1 runs (result unused).
        nc.gpsimd.collective_compute(
            "ReduceScatter",
            OP.add,
            replica_groups=[[0, 1, 2, 3], [4, 5, 6, 7]],
            ins=[t["warm_in_d"].ap()],
            outs=[t["warm_out_d"].ap()],
        )
        # constants that need no DMA: memsets (cheap, no queue dependencies)
        eps_t = persist.tile([128, 1], f32, name="eps", tag="eps")
        nc.vector.memset(eps_t, EPS)
        ones_f = persist.tile([1, 128], f32, name="ones_f", tag="ones_f")
        nc.vector.memset(ones_f, 1.0)
        ones_r = ones_f.bitcast(f32r)

        # small const DMAs on the gpsimd queue (sync queue is reserved for x)
        ident = persist.tile([128, 128], f32r, name="ident", tag="ident")
        nc.gpsimd.dma_start(ident, ident_d.ap().bitcast(f32r))
        g_sb = persist.tile([128, 8], f32, name="g_sb", tag="g_sb")
        nc.gpsimd.dma_start(g_sb, lng_d.ap().rearrange("(c p) -> p c", p=128))
        lnb_sb = persist.tile([128, 8], f32r, name="lnb_sb", tag="lnb_sb")
        nc.gpsimd.dma_start(lnb_sb, lnb_d.ap().rearrange("(c p) -> p c", p=128).bitcast(f32r))
        ctxg_sb = persist.tile([128, 6], f32, name="ctxg_sb", tag="ctxg_sb")
        nc.gpsimd.dma_start(ctxg_sb, ctxg_d.ap().rearrange("(c p) -> p c", p=128))
        ctxb_sb = persist.tile([128, 6], f32r, name="ctxb_sb", tag="ctxb_sb")
        nc.gpsimd.dma_start(ctxb_sb, ctxb_d.ap().rearrange("(c p) -> p c", p=128).bitcast(f32r))

        # null key/value: knull2 rows 0:64 and 64:128 both = null_k (for the two
        # row-packed head positions); nullv2 = [null_v | 1] in bf16 (PV operand).
        knull_st = stat.tile([64, 1], f32, name="knull_st", tag="knull_st")
        nc.gpsimd.dma_start(knull_st, nullkv_d.ap()[0:1, :].rearrange("a b -> b a"))
        knull2 = persist.tile([128, 1], bf16, name="knull2", tag="knull2")
        nc.vector.tensor_copy(knull2[0:64, :], knull_st)
        nc.gpsimd.dma_start(knull2[64:128, :], knull2[0:64, :])
        nullv_f32 = stat.tile([1, 64], f32, name="nullv_f32", tag="nullv_f32")
        nc.gpsimd.dma_start(nullv_f32, nullkv_d.ap()[1:2, :])
        nullv2 = persist.tile([1, 65], bf16, name="nullv2", tag="nullv2")
        nc.vector.tensor_copy(nullv2[0:1, 0:64], nullv_f32)
        nc.vector.memset(nullv2[0:1, 64:65], 1.0)

        # Heavy P0 (weights + context projection), emitted AFTER block-0's LN/transpose
        # chains so the first x tiles hit the DMA queue first.
        wq_sb, wk_sb, wv_sb, wctx_sb, wout_sb = [], [], [], [], []
        cb_q, cb_k = [], []
        cv_row = persist.tile([1, FH], f32r, name="cv_row", tag="cv_row")
        ckvT_sb = persist.tile([128, M_CTX], f32r, name="ckvT", tag="ckvT")
        ck2 = persist.tile([128, M_CTX], bf16, name="ck2", tag="ck2")
        cv_ext = persist.tile([128, 65], bf16, name="cv_ext", tag="cv_ext")

        def emit_p0_heavy(p0sb, psP, psT):
            # weight DMAs: wq/wk on the scalar queue but scheduled after block-0's
            # LN sqrt chain (tile_wait_until); wv/wctx/wout on the gpsimd queue.
            # Per-feature LN gamma folded in on DVE.
            for name, dram, lst, eng, wait in (
                ("wq", wq_d, wq_sb, nc.scalar, 0.005), ("wk", wk_d, wk_sb, nc.scalar, 0.007),
                ("wv", wv_d, wv_sb, nc.gpsimd, 0.0),
            ):
                for c in range(8):
                    w = persist.tile([128, FH], f32r, name=f"{name}{c}", tag=f"{name}{c}")
                    with tc.tile_wait_until(wait, enable=wait > 0):
                        eng.dma_start(w, dram.ap()[128 * c : 128 * (c + 1), :].bitcast(f32r))
                    nc.vector.tensor_scalar_mul(w, w, g_sb[:, c : c + 1])
                    lst.append(w)
            for c in range(6):
                w = persist.tile([128, 2 * D], f32r, name=f"wctx{c}", tag=f"wctx{c}")
                nc.gpsimd.dma_start(w, wctx_d.ap()[128 * c : 128 * (c + 1), :].bitcast(f32r))
                nc.vector.tensor_scalar_mul(w, w, ctxg_sb[:, c : c + 1])
                wctx_sb.append(w)
            for c in range(2):
                wst = p0sb.tile([128, IN], f32, name=f"wout_st{c}", tag="wout_st")
                nc.gpsimd.dma_start(wst, wout_d.ap()[128 * c : 128 * (c + 1), :])
                w = persist.tile([128, IN], bf16, name=f"wout{c}", tag=f"wout{c}")
                nc.vector.tensor_copy(w, wst)
                wout_sb.append(w)
            # LN-beta folded biases: cb[j] = (ln_b @ W')[128j:128j+128] as [128,1]
            for wsb, lst in ((wq_sb, cb_q), (wk_sb, cb_k)):
                for j in range(2):
                    ps = psP.tile([128, 1], f32, name="p0bias", tag="proj")
                    for c in range(8):
                        nc.tensor.matmul(ps, wsb[c][:, 128 * j : 128 * (j + 1)].bitcast(f32),
                                         lnb_sb[:, c : c + 1].bitcast(f32), start=(c == 0), stop=(c == 7))
                    cb = persist.tile([128, 1], f32, name=f"cb{len(lst)}_{id(wsb) % 97}", tag=f"cb{len(cb_q)}_{len(cb_k)}")
                    nc.vector.tensor_copy(cb, ps)
                    lst.append(cb)
            # v bias as a row [1, FH] (added via a K=1 ones matmul)
            psc = psP.tile([1, FH], f32, name="p0cv", tag="proj")
            for c in range(8):
                nc.tensor.matmul(psc, lnb_sb[:, c : c + 1], wv_sb[c], start=(c == 0), stop=(c == 7))
            nc.vector.tensor_copy(cv_row, psc)
            # ---- context projection: ckv^T = W_ctx'.T @ LN(c_emb).T + bias ----
            cemb_sb = p0sb.tile([128, CTX_DIM], f32, name="cemb", tag="cemb")
            nc.gpsimd.dma_start(cemb_sb, cemb_d.ap())
            stc = stat.tile([128, 3, 6], f32, name="stc", tag="stc")
            for i in range(3):
                nc.vector.bn_stats(stc[:, i, :], cemb_sb[:, 256 * i : 256 * (i + 1)])
            mvc = stat.tile([128, 2], f32, name="mvc", tag="mvc")
            nc.vector.bn_aggr(mvc, stc)
            rstd_c = stat.tile([128, 1], f32, name="rstd_c", tag="rstd_c")
            lnv = stat.tile([128, 1], f32, name="lnv", tag="lnv")
            nc.scalar.activation(lnv, mvc[:, 1:2], AF.Sqrt, bias=eps_t[:, 0:1])
            nc.vector.reciprocal_approx_fast(rstd_c, lnv)
            zc = p0sb.tile([128, CTX_DIM], f32r, name="zc", tag="zc")
            nc.vector.tensor_scalar(zc, cemb_sb, mvc[:, 0:1], rstd_c, op0=OP.subtract, op1=OP.mult)
            tpc = psT.tile([128, CTX_DIM], f32r, name="tpc", tag="tp")
            for c in range(6):
                nc.tensor.transpose(tpc[:, 128 * c : 128 * (c + 1)], zc[:, 128 * c : 128 * (c + 1)], ident)
            zcT = p0sb.tile([128, 6, 128], f32r, name="zcT", tag="zcT")
            nc.any.tensor_copy(zcT, tpc.rearrange("p (c w) -> p c w", c=6))
            # bias = (ctx_b @ W_ctx')^T + b_ctx
            psb2 = psP.tile([128, 1], f32, name="p0bias2", tag="proj")
            for c in range(6):
                nc.tensor.matmul(psb2, wctx_sb[c].bitcast(f32), ctxb_sb[:, c : c + 1].bitcast(f32),
                                 start=(c == 0), stop=(c == 5))
            bctx_sb = stat.tile([128, 1], f32, name="bctx_sb", tag="bctx_sb")
            nc.gpsimd.dma_start(bctx_sb, bctx_d.ap().rearrange("(a p) -> p a", p=128))
            ckv_bias = stat.tile([128, 1], f32, name="ckv_bias", tag="ckv_bias")
            nc.vector.tensor_tensor(ckv_bias, psb2, bctx_sb, op=OP.add)
            psk = psP.tile([128, M_CTX], f32, name="psk", tag="proj")
            for c in range(6):
                nc.tensor.matmul(psk, wctx_sb[c], zcT[:, c, :], start=(c == 0), stop=(c == 5))
            nc.scalar.activation(ckvT_sb, psk, AF.Identity, bias=ckv_bias)
            # ck duplicated into both row-halves (for 2-head row packing),
            # converted to bf16 lane-aligned then row-shifted via DMA
            nc.vector.tensor_copy(ck2[0:64, :], ckvT_sb[0:64, :])
            nc.sync.dma_start(ck2[64:128, :], ck2[0:64, :])
            # cv in normal layout [M_CTX, 64] with a ones column -> [128, 65] bf16
            cvT_tmp = p0sb.tile([64, M_CTX], f32r, name="cvT_tmp", tag="cvT_tmp")
            nc.sync.dma_start(cvT_tmp, ckvT_sb[64:128, :])
            ps_cv = psT.tile([128, 64], f32r, name="ps_cv", tag="tp")
            nc.tensor.transpose(ps_cv, cvT_tmp, ident[0:64, 0:64])
            nc.any.tensor_copy(cv_ext[:, 0:64], ps_cv)
            nc.vector.memset(cv_ext[:, 64:65], 1.0)

        # ---------------- persistent activation tensors ----------------
        qT = [persist.tile([128, N], bf16, name=f"qT{j}", tag=f"qT{j}") for j in range(2)]
        kT = [persist.tile([128, N], bf16, name=f"kT{j}", tag=f"kT{j}") for j in range(2)]
        attnT = [persist.tile([128, N], bf16, name=f"attnT{j}", tag=f"attnT{j}") for j in range(2)]
        v_tiles = []
        for i in range(16):
            vt = persist.tile([128, HG, 65], bf16, name=f"v{i}", tag=f"v{i}")
            nc.gpsimd.memset(vt[:, :, 64:65], 1.0)
            v_tiles.append(vt)

        # ---------------- Phase 1: LN(x), transpose, q/k/v projections ----------------
        with tc.tile_pool(name="xp", bufs=3) as xp, \
             tc.tile_pool(name="zp", bufs=2) as zp, \
             tc.tile_pool(name="ztp", bufs=2) as ztp, \
             tc.tile_pool(name="p0sb", bufs=2) as p0sb, \
             tc.tile_pool(name="tpp", bufs=2, space="PSUM") as tpp, \
             tc.tile_pool(name="projp", bufs=2, space="PSUM") as projp, \
             tc.tile_pool(name="vpp", bufs=2, space="PSUM") as vpp:

            def emit_tts(blk):
                zT = ztp.tile([128, 8, BLK], f32r, name="zT", tag="zT")
                for tt in range(4):
                    t0 = BLK * blk + 128 * tt
                    x_t = xp.tile([128, IN], f32, name="x_t", tag="x_t")
                    nc.sync.dma_start(x_t, x_d.ap()[t0 : t0 + 128, :])
                    st = stat.tile([128, 2, 6], f32, name="st", tag="st")
                    nc.vector.bn_stats(st[:, 0, :], x_t[:, 0:512])
                    nc.vector.bn_stats(st[:, 1, :], x_t[:, 512:1024])
                    mv = stat.tile([128, 2], f32, name="mv", tag="mv")
                    nc.vector.bn_aggr(mv, st)
                    lv = stat.tile([128, 1], f32, name="lv", tag="lv")
                    nc.scalar.activation(lv, mv[:, 1:2], AF.Sqrt, bias=eps_t[:, 0:1])
                    rstd = stat.tile([128, 1], f32, name="rstd", tag="rstd")
                    nc.vector.reciprocal_approx_fast(rstd, lv)
                    z_t = zp.tile([128, IN], f32r, name="z_t", tag="z_t")
                    nc.vector.tensor_scalar(z_t, x_t, mv[:, 0:1], rstd, op0=OP.subtract, op1=OP.mult)
                    tp = tpp.tile([128, 1024], f32r, name="tp", tag="tp")
                    for c in range(8):
                        nc.tensor.transpose(tp[:, 128 * c : 128 * (c + 1)], z_t[:, 128 * c : 128 * (c + 1)], ident)
                    nc.vector.tensor_copy(zT[:, :, 128 * tt : 128 * (tt + 1)], tp.rearrange("p (c w) -> p c w", c=8))
                return zT

            def emit_proj(blk, zT):
                # q/k projections (transposed layout), per head-pair j.
                # PSUM->SBUF copy + bias-add fused on the scalar engine.
                for wsb, cbs, dst in ((wq_sb, cb_q, qT), (wk_sb, cb_k, kT)):
                    for j in range(2):
                        ps = projp.tile([128, BLK], f32, name="proj", tag="proj")
                        for c in range(8):
                            nc.tensor.matmul(ps, wsb[c][:, 128 * j : 128 * (j + 1)], zT[:, c, :],
                                             start=(c == 0), stop=(c == 7))
                        nc.scalar.activation(dst[j][:, BLK * blk : BLK * (blk + 1)], ps,
                                             AF.Identity, bias=cbs[j])
                # v projection (normal layout) per 128-token tile
                for tt in range(4):
                    psv = vpp.tile([128, FH], f32, name="psv", tag="psv")
                    for c in range(8):
                        nc.tensor.matmul(psv, zT[:, c, 128 * tt : 128 * (tt + 1)], wv_sb[c],
                                         start=(c == 0), stop=False)
                    nc.tensor.matmul(psv, ones_r, cv_row, start=False, stop=True)
                    vt = v_tiles[4 * blk + tt]
                    for hh in range(HG):
                        nc.scalar.copy(vt[:, hh, 0:64], psv[:, 64 * hh : 64 * (hh + 1)])

            zT0 = emit_tts(0)
            emit_p0_heavy(p0sb, projp, tpp)
            emit_proj(0, zT0)
            for blk in range(1, NBLK):
                zTb = emit_tts(blk)
                emit_proj(blk, zTb)

        # ---------------- Phases 2-4: attention, out-proj, chunked RS + final LN ----------------
        gout_rep = persist.tile([128, IN], f32, name="gout_rep", tag="gout_rep")
        nc.gpsimd.dma_start(gout_rep, outg_d.ap().unsqueeze(0).to_broadcast([128, IN]))
        bout_rep = persist.tile([128, IN], f32, name="bout_rep", tag="bout_rep")
        nc.gpsimd.dma_start(bout_rep, outb_d.ap().unsqueeze(0).to_broadcast([128, IN]))
        with tc.tile_pool(name="wtp", bufs=2) as wtp, \
             tc.tile_pool(name="oddp", bufs=2) as oddp, \
             tc.tile_pool(name="rcpp", bufs=2) as rcpp, \
             tc.tile_pool(name="expnp", bufs=2) as expnp, \
             tc.tile_pool(name="ysb", bufs=3) as ysbp, \
             tc.tile_pool(name="fin", bufs=2) as fin, \
             tc.tile_pool(name="s0p", bufs=3, space="PSUM") as s0p, \
             tc.tile_pool(name="pvp", bufs=2, space="PSUM") as pvp:
            deferred = []
            deferred_fin = []
            fin_steps = []   # previous block's out-proj+RS tiles
            for blk in range(NBLK):
                bsl = slice(BLK * blk, BLK * (blk + 1))
                for pj in range(2):
                    q0 = qT[pj][0:64, bsl]
                    q1 = qT[pj][64:128, bsl]
                    # null-key scores for both heads -> one psum row, one exp
                    expn = expnp.tile([1, 2 * BLK], bf16, name="expn", tag="expn")
                    ps_nl = s0p.tile([1, 2 * BLK], f32, name="ps_nl", tag="ps_s")
                    nc.tensor.matmul(ps_nl[0:1, 0:BLK], knull2[0:64, :], q0, start=True, stop=True)
                    nc.tensor.matmul(ps_nl[0:1, BLK : 2 * BLK], knull2[64:128, :], q1, start=True,
                                     stop=True, tile_position=(64, 0))
                    nc.scalar.activation(expn, ps_nl, AF.Exp, scale=SCALE)
                    # scores -> exp -> PV, pipelined per key tile; both heads share one
                    # [128,1024] scores psum + one exp op (h0 cols 0:512, h1 cols 512:1024).
                    # PV trails a few key tiles behind so PE never head-of-line blocks on exp.
                    ps_pv0 = pvp.tile([65, BLK], f32, name="ps_pv0", tag="ps_pv")
                    ps_pv1 = pvp.tile([65, BLK], f32, name="ps_pv1", tag="ps_pv")

                    def pv_step(kt, wt):
                        lv0 = cv_ext[:, 0:65] if kt == 16 else v_tiles[kt][:, 2 * pj, :]
                        lv1 = cv_ext[:, 0:65] if kt == 16 else v_tiles[kt][:, 2 * pj + 1, :]
                        nc.tensor.matmul(ps_pv0, lv0, wt[:, 0:BLK], start=(kt == 0), stop=False)
                        nc.tensor.matmul(ps_pv1, lv1, wt[:, BLK : 2 * BLK], start=(kt == 0), stop=False)

                    pending = []
                    for kt in range(KT):
                        # do_norm pops before the first pv_step of this pair so the
                        # ps_pv slots are released by its DVE mults in time.
                        if kt == 6 and deferred:
                            deferred.pop(0)()
                        if pj == 0 and kt in (7, 8, 9, 10, 11, 12, 13, 14) and fin_steps:
                            fin_steps.pop(0)()
                        if kt == 14 and pj == 1 and len(deferred_fin) >= 2:
                            deferred_fin.pop(0)()
                        ps_s = s0p.tile([128, 2 * BLK], f32, name="ps_s", tag="ps_s")
                        wt = wtp.tile([128, 2 * BLK], bf16, name="wt", tag="wt", bufs=9)
                        l0 = ck2[0:64, :] if kt == 16 else kT[pj][0:64, 128 * kt : 128 * (kt + 1)]
                        l1 = ck2[64:128, :] if kt == 16 else kT[pj][64:128, 128 * kt : 128 * (kt + 1)]
                        nc.tensor.matmul(ps_s[:, 0:BLK], l0, q0, start=True, stop=True)
                        nc.tensor.matmul(ps_s[:, BLK : 2 * BLK], l1, q1, start=True, stop=True,
                                         tile_position=(64, 0))
                        if len(pending) >= 7:
                            pv_step(*pending.pop(0))
                        if kt in DVE_KTS:
                            nc.vector.tensor_scalar(wt.bitcast(i16), ps_s, EXP_A, EXP_B,
                                                    op0=OP.mult, op1=OP.add)
                        else:
                            nc.scalar.activation(wt, ps_s, AF.Exp, scale=SCALE)
                        pending.append((kt, wt))
                    for args in pending:
                        pv_step(*args)
                    nc.tensor.matmul(ps_pv0, nullv2[0:1, :], expn[0:1, 0:BLK], start=False, stop=True)
                    nc.tensor.matmul(ps_pv1, nullv2[0:1, :], expn[0:1, BLK : 2 * BLK], start=False, stop=True)

                    # normalize: attnT = pv[0:64] * broadcast(1/denominator).  The
                    # denominator row goes PSUM p64 -> SBUF p64 (ACT) -> DRAM -> SBUF
                    # [64, BLK] partition-broadcast (DMA), reciprocal + multiply on DVE
                    # (no PE involvement at all).
                    rcps = []
                    for h, ps_pv in ((0, ps_pv0), (1, ps_pv1)):
                        didx = (blk * 2 + pj) * 2 + h
                        den64 = rcpp.tile([65, BLK], f32, name="den64", tag="den64")
                        nc.vector.tensor_copy(den64[64:65, :], ps_pv[64:65, :])
                        nc.sync.dma_start(dden_d.ap()[didx : didx + 1, :], den64[64:65, :])
                        den_b = rcpp.tile([64, BLK], f32, name="den_b", tag="den_b")
                        nc.sync.dma_start(den_b, dden_d.ap()[didx, :].unsqueeze(0).to_broadcast([64, BLK]))
                        rb = rcpp.tile([64, BLK], f32, name="rb", tag="rb")
                        nc.vector.reciprocal_approx_fast(rb, den_b)
                        rcps.append(rb)

                    def do_norm(pj=pj, bsl=bsl, pvs=(ps_pv0, ps_pv1), rcps=tuple(rcps)):
                        for h, (ps_pv, rb) in enumerate(zip(pvs, rcps)):
                            if h == 0:
                                nc.vector.tensor_tensor(attnT[pj][0:64, bsl], ps_pv[0:64, :], rb, op=OP.mult)
                            else:
                                tmp = oddp.tile([64, BLK], bf16, name="odd", tag="odd")
                                nc.vector.tensor_tensor(tmp, ps_pv[0:64, :], rb, op=OP.mult)
                                nc.sync.dma_start(attnT[pj][64:128, bsl], tmp)

                    deferred.append(do_norm)
                    if pj == 0 and fin_steps:
                        # previous block's remaining out-proj tiles
                        while fin_steps:
                            fin_steps.pop(0)()

                # out-projection + per-tile RS for this block, deferred into the
                # next block's pair-0 key loop so the block boundary never stalls
                # on the pair-1 normalize chain
                def make_ostep(blk=blk, tt4=0):
                    def ostep(tt4=tt4, blk=blk):
                        tt = 4 * blk + tt4
                        y_sb = ysbp.tile([128, IN], bf16, name="y_sb", tag="y_sb")
                        for nh in range(2):
                            ps_y = s0p.tile([128, 512], f32, name="ps_y", tag="ps_s")
                            for c in range(2):
                                nc.tensor.matmul(ps_y, attnT[c][:, 128 * tt : 128 * (tt + 1)],
                                                 wout_sb[c][:, 512 * nh : 512 * (nh + 1)],
                                                 start=(c == 0), stop=(c == 1))
                            nc.vector.tensor_copy(y_sb[:, 512 * nh : 512 * (nh + 1)], ps_y)
                        nc.sync.dma_start(ypart_d[blk].ap()[128 * tt4 : 128 * (tt4 + 1), :], y_sb)
                        if tt4 == 3:
                            # block ReduceScatter: rank r receives rows [128r, 128(r+1))
                            nc.gpsimd.collective_compute(
                                "ReduceScatter",
                                OP.add,
                                replica_groups=[[0, 1, 2, 3], [4, 5, 6, 7]],
                                ins=[ypart_d[blk].ap()],
                                outs=[yred_d[blk].ap()],
                            )
                            deferred_fin.append(make_final_ln(blk))
                    return ostep

                fin_steps = [make_ostep(blk, t4) for t4 in range(4)]

                # final LN per block on the received 4x32 rows -- stats + normalize
                # on the ACT engine (accumulator row-sums, fused scale/bias), quake
                # rsqrt small chain on DVE, gamma/beta application on Pool, so no
                # single engine queue is head-blocked waiting on the collective.
                def make_final_ln(blk=blk):
                  def final_ln(blk=blk):
                    yr_b = fin.tile([128, IN], bf16, name="yr_b", tag="yr_b")
                    nc.gpsimd.dma_start(yr_b, yred_d[blk].ap())
                    yr = fin.tile([128, IN], f32, name="yr", tag="yr")
                    rsum = stat.tile([128, 1], f32, name="rsum", tag="rsum")
                    nc.scalar.activation(yr, yr_b, AF.Identity, accum_out=rsum)
                    ysq = fin.tile([128, IN], f32, name="ysq", tag="ysq")
                    rsumsq = stat.tile([128, 1], f32, name="rsumsq", tag="rsumsq")
                    nc.scalar.activation(ysq, yr_b, AF.Square, accum_out=rsumsq)
                    mean = stat.tile([128, 1], f32, name="mean", tag="mean")
                    nc.vector.tensor_scalar_mul(mean, rsum, 1.0 / IN)
                    # vpe = E[y^2] - mean^2 + eps
                    negm2 = stat.tile([128, 1], f32, name="negm2", tag="negm2")
                    nc.vector.scalar_tensor_tensor(negm2, mean, -1.0, mean, op0=OP.mult, op1=OP.mult)
                    varr = stat.tile([128, 1], f32, name="varr", tag="varr")
                    nc.vector.scalar_tensor_tensor(varr, rsumsq, 1.0 / IN, negm2, op0=OP.mult, op1=OP.add)
                    vpe = stat.tile([128, 1], f32, name="vpe", tag="vpe")
                    nc.vector.tensor_scalar_add(vpe, varr, EPS)
                    # rstd = (var+eps)^-1/2 on DVE only (quake seed + 2 Newton
                    # steps) -- keeps the ACT engine's exp table resident.
                    rstd = stat.tile([128, 1], f32, name="rstdq", tag="rstdq")
                    tq = stat.tile([128, 1], f32, name="tq", tag="tq")
                    nc.vector.tensor_scalar(rstd.bitcast(i32), vpe.bitcast(i32), 1, -1,
                                            op0=OP.logical_shift_right, op1=OP.bitwise_xor)
                    nc.vector.tensor_scalar_add(rstd.bitcast(i32), rstd.bitcast(i32), 0x5F3759E0)
                    for _ in range(2):
                        nc.vector.tensor_tensor(tq, rstd, rstd, op=OP.mult)
                        nc.vector.tensor_tensor(tq, tq, vpe, op=OP.mult)
                        nc.vector.tensor_scalar(tq, tq, -0.5, 1.5, op0=OP.mult, op1=OP.add)
                        nc.vector.tensor_tensor(rstd, rstd, tq, op=OP.mult)
                    negmur = stat.tile([128, 1], f32, name="negmur", tag="negmur")
                    nc.vector.scalar_tensor_tensor(negmur, mean, -1.0, rstd, op0=OP.mult, op1=OP.mult)
                    # zf = yr*rstd - mean*rstd (one fused ACT op), then *g +b on Pool
                    zf = fin.tile([128, IN], f32, name="zf", tag="zf")
                    nc.scalar.activation(zf, yr, AF.Identity, scale=rstd[:, 0:1], bias=negmur[:, 0:1])
                    nc.vector.tensor_tensor(zf, zf, gout_rep, op=OP.mult)
                    of = fin.tile([128, IN], f32, name="of", tag="of")
                    nc.vector.tensor_tensor(of, zf, bout_rep, op=OP.add)
                    nc.gpsimd.dma_start(y_out_d.ap()[128 * blk : 128 * (blk + 1), :], of)
                  return final_ln

            # tail: last block's normalize, out-proj + RS, remaining final LNs
            while deferred:
                deferred.pop(0)()
            while fin_steps:
                fin_steps.pop(0)()
            while deferred_fin:
                deferred_fin.pop(0)()


def shard_inputs(inputs):
    """Split full inputs into 8 per-core input maps."""
    x = np.ascontiguousarray(np.asarray(inputs["x"], dtype=np.float32))
    c_emb = np.ascontiguousarray(np.asarray(inputs["c_emb"], dtype=np.float32))
    W_q = np.asarray(inputs["W_q"], np.float32).reshape(IN, H, D)
    W_kv = np.asarray(inputs["W_kv"], np.float32).reshape(IN, 2, H, D)
    W_out = np.asarray(inputs["W_out"], np.float32).reshape(H, D, IN)
    common = {
        "const_ident": np.eye(128, dtype=np.float32),
        "wctx": np.ascontiguousarray(np.asarray(inputs["W_ctx"], np.float32)),
        "nullkv": np.ascontiguousarray(np.asarray(inputs["null_kv"], np.float32)),
        "ln_g": np.ascontiguousarray(np.asarray(inputs["ln_g"], np.float32)),
        "ln_b": np.ascontiguousarray(np.asarray(inputs["ln_b"], np.float32)),
        "ctx_g": np.ascontiguousarray(np.asarray(inputs["ctx_ln_g"], np.float32)),
        "ctx_b": np.ascontiguousarray(np.asarray(inputs["ctx_ln_b"], np.float32)),
        "b_ctx": np.ascontiguousarray(np.asarray(inputs["b_ctx"], np.float32)),
        "out_g": np.ascontiguousarray(np.asarray(inputs["out_ln_g"], np.float32)),
        "out_b": np.ascontiguousarray(np.asarray(inputs["out_ln_b"], np.float32)),
    }
    in_maps = []
    for c in range(NCORES):
        b, g = c // 4, c % 4
        hs = slice(HG * g, HG * (g + 1))
        in_maps.append({
            "x_loc": x[b],
            "cemb_loc": c_emb[b],
            "wq_loc": np.ascontiguousarray(W_q[:, hs].reshape(IN, FH)),
            "wk_loc": np.ascontiguousarray(W_kv[:, 0, hs].reshape(IN, FH)),
            "wv_loc": np.ascontiguousarray(W_kv[:, 1, hs].reshape(IN, FH)),
            "wout_loc": np.ascontiguousarray(W_out[hs].reshape(FH, IN)),
            **common,
        })
    return in_maps


def unshard(results):
    out = np.empty((B, N, IN), np.float32)
    for c in range(NCORES):
        b, r = c // 4, c % 4
        y = results[c]["y_out"]
        for blk in range(NBLK):
            t0 = BLK * blk + 128 * r
            out[b, t0 : t0 + 128, :] = y[128 * blk : 128 * (blk + 1)]
    return out


_CACHE = {}


def kernel(**inputs) -> np.ndarray:
    from concourse.bass_utils import run_bass_kernel_spmd

    if "nc" not in _CACHE:
        _CACHE["nc"] = build_program()
    nc = _CACHE["nc"]
    in_maps = shard_inputs(inputs)
    res = run_bass_kernel_spmd(nc, in_maps, list(range(NCORES))).results
    return unshard(res)


if __name__ == "__main__":
    nc = build_program()
    print("program built OK;",
          sum(1 for _ in nc.inst_map), "instructions")
